# revision 24
# baseline (speedup 1.0000x reference)
"""Trainium2 Bass kernel for the Critic (gnn_message_passing) problem.

Math (per sample b):
  wg   = W_w @ g + W_b                                  [32]
  score_l = lrelu(x_l . v + c_b)   with v = U_w^T a2,
        c_b = a1.wg + att_b + U_b.a2
  score_g = lrelu((a1+a2).wg + att_b)
  total = score_g + sum_l score_l
  l_part = (U_w @ m_b + U_b * s_b) / total,  m_b = sum_l score_l x_l
  g_part = (score_g / total) * wg
  sa = [relu(g_part); relu(l_part); action]
  q_h = l3 @ relu(l2 @ relu(l1 @ sa + b1) + b2) + b3   (two heads)

Layout (per core, pure data parallel x8, B_LOC = 512 samples):
  - local_states streamed in 32 fp32 chunks [128 part, 25*128]; partition p
    holds 25 consecutive tokens of sample p//8. Chunk loads alternate
    between the two HWDGE rings (qSP / qAct), software-pipelined PF chunks
    ahead; setup loads are batched into few large DMAs so dispatch never
    congests the rings.
  - t = x.v via ONE custom DVE op per chunk: a scan(ADD, Src0*Src1)
    cumulative sum whose OUTPUT access pattern has inner stride 0 over
    each 128-elem segment, so only each segment's last value (the
    per-token dot product boundary) lands in a tiny [128, 26] tile.
    A second custom op computes score16 = fp16(lrelu(E[j+1]-E[j]+c)).
  - NO fp16 copy of x: the PE m-pass reads x as bf16 bitcast views
    (high halfword of each fp32 = truncated bf16), stride-2 APs.
    Validated: score fp16 + x bf16-trunc -> ~9.4e-3 final rel err.
  - scorem = score16 (x) m16 on DVE (GPSIMD stalls behind DVE's SBUF
    port during 2-src custom ops, so GPSIMD is kept out of the stream).
  - m accumulated on PE (25 matmuls of [128,16] into PSUM per chunk);
    s via one m16-stationary matmul + ACT accumulate.
  - Transposes for phase C are emitted mid-stream (after chunk 5); phase C
    (wg/sg + combine + head MLPs) runs per 128-sample block inside the
    stream, since a block's normalization only needs its own 8 chunks.
"""
import os
import sys

sys.path.insert(0, "/opt/trn_rl_repo")

from contextlib import ExitStack

import numpy as np

import concourse.bass as bass
import concourse.tile as tile
from concourse import bacc
from concourse import mybir
from concourse import dve_ops as DO
from concourse.dve_ops import TENSOR_TENSOR_REDUCE as CUSTOM_TTR
from concourse.dve_spec import (Spec, Src0, Src1, C0, C1, lower, AluOp, scan,
                                maxx, _has_src1)
from concourse.dve_uop import DveOpSpec

F32 = mybir.dt.float32
F16 = mybir.dt.float16
BF16 = mybir.dt.bfloat16
AF = mybir.AluOpType
IDENT = mybir.ActivationFunctionType.Identity
RELU = mybir.ActivationFunctionType.Relu

G_DIM, L_DIM, A_DIM, HID = 256, 128, 64, 32
B, L = 4096, 200
NCORES = 8
B_LOC = B // NCORES          # 512 samples per core
J = 25                       # tokens per partition per chunk
SPC = 16                     # samples per chunk (128 partitions / 8 per sample)
PPS = L // J                 # partitions per sample = 8
NCHUNK = B_LOC // SPC        # 32 chunks
NB = B_LOC // 128            # 128-sample blocks
CPB = NCHUNK // NB           # chunks per block = 8
NTOK = J * 128               # free-dim elements per chunk (3200)
PF = 8                       # chunk DMA prefetch distance
XBUFS = 11                   # x_ch ring depth (>= PF + 2)
TAIL_CH = 5                  # emit phase-A-tail transposes after this chunk


def _register_dve_op(name, spec, subdim=False):
    if name in DO._SUB_OPCODE_FOR_NAME:
        return next(op for op in DO.OPS if op.name == name)
    row = DO._CUSTOM_DVE_ROW_BASE + len(DO.OPS)
    assert row < 0x20
    DO._SUB_OPCODE_FOR_NAME[name] = row
    shas = {}
    for ver in ("v3", "v4"):
        shas[ver] = DveOpSpec(name=name, opcode=row, uops=lower(spec, ver=ver),
                              rd1_en=_has_src1(spec)).sha(ver)
    op = DO.DveOp(name, spec, subdim=subdim, uops_sha=shas)
    DO.OPS.append(op)
    DO.CUSTOM_DVE_SPECS[name] = spec
    return op


def _ref_cumsum_mul(in0, in1, c0, c1, c2):
    return np.cumsum(in0.astype(np.float32) * np.asarray(in1, np.float32),
                     axis=-1, dtype=np.float32)


def _ref_score(in0, in1, c0, c1, c2):
    dd = in0.astype(np.float32) - np.asarray(in1, np.float32) + c0
    return np.maximum(dd, dd * c1)


CUMSUM_MUL = _register_dve_op(
    "CUMSUM_MUL_ANT",
    Spec(body=scan(AluOp.ADD, Src0 * Src1), reference=_ref_cumsum_mul))

SCORE_OP = _register_dve_op(
    "DIFF_BIAS_LRELU_ANT",
    Spec(body=maxx((Src0 - Src1) + C0, ((Src0 - Src1) + C0) * C1),
         reference=_ref_score))


def build_bass(b_loc=B_LOC):
    tok = b_loc * L
    nc = bacc.Bacc()

    ls = nc.dram_tensor("local_states", [tok, L_DIM], F32, kind="ExternalInput")
    gs = nc.dram_tensor("global_states", [b_loc, G_DIM], F32, kind="ExternalInput")
    ac = nc.dram_tensor("actions", [b_loc, A_DIM], F32, kind="ExternalInput")
    Ww = nc.dram_tensor("W_w", [HID, G_DIM], F32, kind="ExternalInput")
    Wb = nc.dram_tensor("W_b", [HID], F32, kind="ExternalInput")
    Uw = nc.dram_tensor("U_w", [HID, L_DIM], F32, kind="ExternalInput")
    Ub = nc.dram_tensor("U_b", [HID], F32, kind="ExternalInput")
    attw = nc.dram_tensor("att_w", [1, 2 * HID], F32, kind="ExternalInput")
    attb = nc.dram_tensor("att_b", [1], F32, kind="ExternalInput")
    heads = []
    for h, names in enumerate((("l1", "l2", "l3"), ("l4", "l5", "l6"))):
        w1 = nc.dram_tensor(f"{names[0]}_w", [256, 128], F32, kind="ExternalInput")
        b1 = nc.dram_tensor(f"{names[0]}_b", [256], F32, kind="ExternalInput")
        w2 = nc.dram_tensor(f"{names[1]}_w", [256, 256], F32, kind="ExternalInput")
        b2 = nc.dram_tensor(f"{names[1]}_b", [256], F32, kind="ExternalInput")
        w3 = nc.dram_tensor(f"{names[2]}_w", [1, 256], F32, kind="ExternalInput")
        b3 = nc.dram_tensor(f"{names[2]}_b", [1], F32, kind="ExternalInput")
        heads.append((w1, b1, w2, b2, w3, b3))
    m16_d = nc.dram_tensor("m16", [128, SPC], F16, kind="ExternalInput")
    esel_d = nc.dram_tensor("esel", [128, PPS * 128], F32, kind="ExternalInput")
    out_d = nc.dram_tensor("out", [2, b_loc], F32, kind="ExternalOutput")

    with tile.TileContext(nc) as tc, ExitStack() as ctx:
        P = ctx.enter_context(tc.tile_pool(name="persist", bufs=1))
        scratch = ctx.enter_context(tc.tile_pool(name="scratch", bufs=2))
        ps_t = ctx.enter_context(tc.tile_pool(name="ps_t", bufs=2, space="PSUM"))

        # ---------------- Phase A: setup --------------------------------
        from concourse.masks import make_identity

        ident = P.tile([128, 128], F32, tag="ident")
        make_identity(nc, ident[:, :])
        ones_row = P.tile([1, 128], F32, tag="onesr")
        nc.vector.memset(ones_row[:, :], 1.0)

        # --- setup DMAs, sync ring (small weights + globals) ---
        m16_sb = P.tile([128, SPC], F16, tag="m16")
        nc.sync.dma_start(m16_sb[:, :], m16_d[:, :])
        esel = P.tile([128, PPS * 128], F32, tag="esel")
        nc.sync.dma_start(esel[:, :], esel_d[:, :])
        Ww_sb = P.tile([HID, G_DIM], F32, tag="Ww")
        nc.sync.dma_start(Ww_sb[:, :], Ww[:, :])
        Wb_sb = P.tile([HID, 1], F32, tag="Wb")
        nc.sync.dma_start(Wb_sb[:, :], Wb[:][:, None])
        Uw_sb = P.tile([HID, L_DIM], F32, tag="Uw")
        nc.sync.dma_start(Uw_sb[:, :], Uw[:, :])
        Ub_col = P.tile([HID, 1], F32, tag="Ubc")
        nc.sync.dma_start(Ub_col[:, :], Ub[:][:, None])
        Ub_row = P.tile([1, HID], F32, tag="Ubr")
        nc.sync.dma_start(Ub_row[:, :], Ub[:][None, :])
        a1_sb = P.tile([HID, 1], F32, tag="a1")
        nc.sync.dma_start(a1_sb[:, :], attw[0, 0:HID][:, None])
        a2_sb = P.tile([HID, 1], F32, tag="a2")
        nc.sync.dma_start(a2_sb[:, :], attw[0, HID:2 * HID][:, None])
        attb_sb = P.tile([1, 1], F32, tag="attb")
        nc.sync.dma_start(attb_sb[:, :], attb[:][None, :])
        # all 512 global states in one DMA: partition p <- sample bb*128+p
        g_all = P.tile([128, NB * G_DIM], F32, tag="gall")
        nc.sync.dma_start(
            g_all[:, :].rearrange("p (bb g) -> p bb g", g=G_DIM),
            gs[:, :].rearrange("(bb p) g -> p bb g", p=128))

        # --- setup DMAs, scalar ring (head weights + actions, batched) ---
        head_dma = []
        for hh, (w1, b1, w2, b2, w3, b3) in enumerate(heads):
            w1n = P.tile([128, 256], F32, tag=f"w1n{hh}")
            nc.scalar.dma_start(
                w1n[:, :].rearrange("p (r d) -> p r d", d=128),
                w1[:, :].rearrange("(r p) d -> p r d", p=128))
            w2n = P.tile([128, 512], F32, tag=f"w2n{hh}")
            nc.scalar.dma_start(
                w2n[:, :].rearrange("p (r d) -> p r d", d=256),
                w2[:, :].rearrange("(r p) d -> p r d", p=128))
            w3T = P.tile([128, 2], F32, tag=f"w3T{hh}")
            nc.scalar.dma_start(w3T[:, :],
                                w3[0, :].rearrange("(k p) -> p k", p=128))
            b1c = P.tile([128, 2], F32, tag=f"b1c{hh}")
            nc.scalar.dma_start(b1c[:, :],
                                b1[:].rearrange("(r p) -> p r", p=128))
            b2c = P.tile([128, 2], F32, tag=f"b2c{hh}")
            nc.scalar.dma_start(b2c[:, :],
                                b2[:].rearrange("(r p) -> p r", p=128))
            b3c = P.tile([1, 1], F32, tag=f"b3c{hh}")
            nc.scalar.dma_start(b3c[:, :], b3[:][None, :])
            head_dma.append((w1n, w2n, w3T, b1c, b2c, b3c))
        a_all = P.tile([128, NB * A_DIM], F32, tag="aall")
        nc.scalar.dma_start(
            a_all[:, :].rearrange("p (bb a) -> p bb a", a=A_DIM),
            ac[:, :].rearrange("(bb p) a -> p bb a", p=128))

        # --- v_flat [128, 3200] fp32 (PE + DVE copies only) ---
        v_ps = ps_t.tile([1, L_DIM], F32, tag="tps")
        nc.tensor.matmul(out=v_ps[:, :], lhsT=a2_sb[:, :], rhs=Uw_sb[:, :])
        v_row = P.tile([1, L_DIM], F32, tag="vrow")
        nc.vector.tensor_copy(v_row[:, :], v_ps[:, :])
        vrep_ps = ps_t.tile([128, 128], F32, tag="tps")
        nc.tensor.matmul(out=vrep_ps[:, :], lhsT=ones_row[:, :], rhs=v_row[:, :])
        v_rep = P.tile([128, 128], F32, tag="vrep")
        nc.vector.tensor_copy(v_rep[:, :], vrep_ps[:, :])

        # --- c_all [128, NCHUNK] ---
        u_ps = ps_t.tile([1, G_DIM], F32, tag="tps")
        nc.tensor.matmul(out=u_ps[:, :], lhsT=a1_sb[:, :], rhs=Ww_sb[:, :])
        u_row = P.tile([1, G_DIM], F32, tag="urow")
        nc.vector.tensor_copy(u_row[:, :], u_ps[:, :])
        urep_ps = ps_t.tile([128, G_DIM], F32, tag="tps")
        nc.tensor.matmul(out=urep_ps[:, :], lhsT=ones_row[:, :], rhs=u_row[:, :])
        u_rep = P.tile([128, G_DIM], F32, tag="urep")
        nc.vector.tensor_copy(u_rep[:, :], urep_ps[:, :])
        # c_col4 off the DVE: multiply on GPSIMD, reduce via ACT accumulate
        # (both engines idle pre-stream; shortens the DVE queue before cum0)
        c_col4 = P.tile([128, NB], F32, tag="ccol4")
        for bb in range(NB):
            junkA = scratch.tile([128, G_DIM], F32, tag="junkA",
                                 name=f"junkA{bb}")
            nc.gpsimd.tensor_tensor(
                out=junkA[:, :],
                in0=g_all[:, bb * G_DIM:(bb + 1) * G_DIM],
                in1=u_rep[:, :], op=AF.mult)
            junkB = scratch.tile([128, G_DIM], F32, tag="junkB",
                                 name=f"junkB{bb}")
            nc.scalar.activation(junkB[:, :], junkA[:, :], IDENT,
                                 accum_out=c_col4[:, bb:bb + 1])
        uba2_ps = ps_t.tile([1, 1], F32, tag="tps")
        nc.tensor.matmul(out=uba2_ps[:, :], lhsT=Ub_col[:, :], rhs=a2_sb[:, :],
                         start=True, stop=False, skip_group_check=True)
        nc.tensor.matmul(out=uba2_ps[:, :], lhsT=Wb_sb[:, :], rhs=a1_sb[:, :],
                         start=False, stop=True, skip_group_check=True)
        cconst = P.tile([1, 1], F32, tag="cconst")
        nc.vector.tensor_tensor(out=cconst[:, :], in0=uba2_ps[:, :],
                                in1=attb_sb[:, :], op=AF.add)
        cc128_ps = ps_t.tile([128, 1], F32, tag="tps")
        nc.tensor.matmul(out=cc128_ps[:, :], lhsT=ones_row[0:1, :],
                         rhs=cconst[:, :])
        cc128 = P.tile([128, 1], F32, tag="cc128")
        nc.vector.tensor_copy(cc128[:, :], cc128_ps[:, :])
        call_ps = ps_t.tile([128, NCHUNK], F32, tag="tps")
        for r in range(PPS):
            nc.tensor.matmul(out=call_ps[:, r:NCHUNK:PPS],
                             lhsT=esel[:, r * 128:(r + 1) * 128],
                             rhs=c_col4[:, :], skip_group_check=True)
        c_all = P.tile([128, NCHUNK], F32, tag="call")
        nc.scalar.activation(c_all[:, :], call_ps[:, :], IDENT,
                             bias=cc128[:, :])

        a12 = P.tile([HID, 1], F32, tag="a12")
        nc.vector.tensor_tensor(out=a12[:, :], in0=a1_sb[:, :], in1=a2_sb[:, :],
                                op=AF.add)

        # ---------------- pools for stream + phase C ----------------
        xpool = ctx.enter_context(tc.tile_pool(name="xchunk", bufs=XBUFS))
        cumpool = ctx.enter_context(tc.tile_pool(name="cump", bufs=3))
        scpool = ctx.enter_context(tc.tile_pool(name="score", bufs=3))
        smpool = ctx.enter_context(tc.tile_pool(name="scorem", bufs=2))
        jpool = ctx.enter_context(tc.tile_pool(name="junk", bufs=2))
        ps_m = ctx.enter_context(tc.tile_pool(name="ps_m", bufs=2, space="PSUM"))
        ps_s = ctx.enter_context(tc.tile_pool(name="ps_s", bufs=1, space="PSUM"))
        ps_c = ctx.enter_context(tc.tile_pool(name="ps_c", bufs=3, space="PSUM"))

        mT = P.tile([L_DIM, b_loc], F32, tag="mT")
        s_colT = P.tile([SPC, NCHUNK], F32, tag="scolT")
        saT = P.tile([128, b_loc], F32, tag="saT")

        # filled by the phase-A tail (emitted mid-stream)
        gT = [P.tile([128, b_loc], F32, tag=f"gT{g}", name=f"gT{g}")
              for g in range(G_DIM // 128)]
        WwT = [P.tile([128, HID], F32, tag=f"WwT{g}", name=f"WwT{g}")
               for g in range(G_DIM // 128)]
        UwT = P.tile([L_DIM, HID], F32, tag="UwT")
        head_sb = []

        x_tiles = [None] * NCHUNK

        def transpose_to_sbuf(dst_ap, src_ap):
            pp, ff = src_ap.shape
            t_ps = ps_t.tile([128, 128], F32, tag="tps")
            nc.tensor.transpose(t_ps[0:ff, 0:pp], src_ap, ident[0:pp, 0:pp])
            nc.scalar.copy(dst_ap, t_ps[0:ff, 0:pp])

        def issue_dma(ch):
            x_ch = xpool.tile([128, NTOK], F32, tag="xch")
            src = ls[ch * NTOK:(ch + 1) * NTOK, :]
            dma_eng = nc.sync if ch % 2 == 0 else nc.scalar
            dma_eng.dma_start(
                x_ch[:, :], src.rearrange("(p j) d -> p (j d)", p=128))
            x_tiles[ch] = x_ch

        def compute_chunk(ch):
            x_ch = x_tiles[ch]
            # boundary-only cumsum(x * v): inner write stride 0, so each
            # 128-segment's final running sum lands at cum[:, 1+j].
            cum = cumpool.tile([128, 32], F32, tag="cum")
            nc.gpsimd.memset(cum[:, 0:1], 0.0)
            nc.vector._custom_dve(
                CUMSUM_MUL,
                out=cum[:, 1:J + 1, None].broadcast_to((128, J, 128)),
                in0=x_ch[:, :].rearrange("p (j d) -> p j d", d=128),
                in1=v_rep[:, None, :].broadcast_to((128, J, 128)))

            # score16[p, j] = fp16(lrelu(E[j+1] - E[j] + c))
            score16 = scpool.tile([128, J], F16, tag="sc")
            nc.vector._custom_dve(SCORE_OP, out=score16[:, :],
                                  in0=cum[:, 1:J + 1],
                                  in1=cum[:, 0:J],
                                  s0=c_all[:, ch:ch + 1], s1=0.01)

            # scorem[p, (j,s)] = score16[p,j] * m16[p,s]
            scorem = smpool.tile([128, J * SPC], F16, tag="sm")
            sm3 = scorem[:, :].rearrange("p (j s) -> p j s", s=SPC)
            nc.vector.tensor_tensor(
                out=sm3[:, :, :],
                in0=score16[:, :, None].broadcast_to((128, J, SPC)),
                in1=m16_sb[:, None, :].broadcast_to((128, J, SPC)),
                op=AF.mult)

            # mT_chunk [128 feat, 16] accumulated over the 25 j-tiles;
            # lhsT is the bf16 high-halfword view of the fp32 x tile.
            mT_ps = ps_m.tile([L_DIM, SPC], F32, tag="mps")
            for j in range(J):
                xbj = x_ch[:, j * 128:(j + 1) * 128].bitcast(BF16)[:, 1:256:2]
                nc.tensor.matmul(out=mT_ps[:, :],
                                 lhsT=xbj,
                                 rhs=scorem[:, j * SPC:(j + 1) * SPC],
                                 start=(j == 0), stop=(j == J - 1))
            nc.scalar.copy(mT[:, ch * SPC:(ch + 1) * SPC], mT_ps[:, :])

            # s[s] = sum_{p,j} score16[p,j]*ind(p//8==s)  -> s_colT[:, ch]
            s_ps = ps_s.tile([SPC, J], F32, tag="sps")
            nc.tensor.matmul(out=s_ps[:, :], lhsT=m16_sb[:, :],
                             rhs=score16[:, :])
            junk_s = jpool.tile([SPC, J], F32, tag="jks")
            nc.scalar.activation(junk_s[:, :], s_ps[:, :], IDENT,
                                 accum_out=s_colT[:, ch:ch + 1])

        def phase_a_tail():
            for bb in range(NB):
                for g in range(G_DIM // 128):
                    transpose_to_sbuf(
                        gT[g][:, bb * 128:(bb + 1) * 128],
                        g_all[:, bb * G_DIM + g * 128:bb * G_DIM + (g + 1) * 128])
            for g in range(G_DIM // 128):
                transpose_to_sbuf(WwT[g][:, :], Ww_sb[:, g * 128:(g + 1) * 128])
            transpose_to_sbuf(UwT[:, :], Uw_sb[:, :])
            for bb in range(NB):
                transpose_to_sbuf(
                    saT[2 * HID:2 * HID + A_DIM, bb * 128:(bb + 1) * 128],
                    a_all[:, bb * A_DIM:(bb + 1) * A_DIM])
            for hh, (w1n, w2n, w3T, b1c, b2c, b3c) in enumerate(head_dma):
                w1T = P.tile([128, 256], F32, tag=f"w1T{hh}")
                for rh in range(2):
                    transpose_to_sbuf(w1T[:, rh * 128:(rh + 1) * 128],
                                      w1n[:, rh * 128:(rh + 1) * 128])
                w2T = [P.tile([128, 256], F32, tag=f"w2T{hh}_{kh}",
                              name=f"w2T{hh}_{kh}") for kh in range(2)]
                for rh in range(2):
                    for kh in range(2):
                        transpose_to_sbuf(
                            w2T[kh][:, rh * 128:(rh + 1) * 128],
                            w2n[:, rh * 256 + kh * 128:rh * 256 + (kh + 1) * 128])
                head_sb.append((w1T, w2T, w3T, b1c, b2c, b3c))

        def phase_c_block(bb):
            sl = slice(bb * 128, (bb + 1) * 128)
            ch0 = bb * CPB
            # wg block [32, 128] + sg row
            wg_ps = ps_c.tile([HID, 128], F32, tag="cps")
            for g in range(G_DIM // 128):
                nc.tensor.matmul(out=wg_ps[:, :], lhsT=WwT[g][:, :],
                                 rhs=gT[g][:, sl],
                                 start=(g == 0), stop=(g == G_DIM // 128 - 1))
            wg_sb = scratch.tile([HID, 128], F32, tag="wgsb")
            nc.scalar.activation(wg_sb[:, :], wg_ps[:, :], IDENT,
                                 bias=Wb_sb[:, :])
            sg_ps = ps_c.tile([1, 128], F32, tag="cps")
            nc.tensor.matmul(out=sg_ps[:, :], lhsT=a12[:, :], rhs=wg_sb[:, :])
            sg_lin = scratch.tile([1, 128], F32, tag="sglin")
            nc.scalar.activation(sg_lin[:, :], sg_ps[:, :], IDENT,
                                 bias=attb_sb[:, :])
            sg_raw = scratch.tile([1, 128], F32, tag="sgraw")
            nc.vector.scalar_tensor_tensor(out=sg_raw[:, :], in0=sg_lin[:, :],
                                           scalar=0.01, in1=sg_lin[:, :],
                                           op0=AF.mult, op1=AF.max)

            # s_row block [1, 128]: sample ch*16+s -> col (ch-ch0)*16+s
            srow_ps = ps_c.tile([1, 128], F32, tag="cps")
            for s in range(SPC):
                nc.tensor.matmul(out=srow_ps[0:1, s * CPB:(s + 1) * CPB],
                                 lhsT=ident[0:SPC, s:s + 1],
                                 rhs=s_colT[:, ch0:ch0 + CPB],
                                 skip_group_check=True)
            s_row = scratch.tile([1, 128], F32, tag="srow")
            nc.scalar.copy(
                s_row[0:1, :].rearrange("one (c s) -> one c s", s=SPC),
                srow_ps[0:1, :].rearrange("one (s c) -> one c s", s=SPC))

            total = scratch.tile([1, 128], F32, tag="total")
            nc.vector.tensor_tensor(out=total[:, :], in0=sg_raw[:, :],
                                    in1=s_row[:, :], op=AF.add)
            recip = scratch.tile([1, 128], F32, tag="recip")
            nc.vector.reciprocal_approx_fast(recip[:, :], total[:, :])
            gn_row = scratch.tile([1, 128], F32, tag="gn")
            nc.vector.tensor_tensor(out=gn_row[:, :], in0=sg_raw[:, :],
                                    in1=recip[:, :], op=AF.mult)

            r32_ps = ps_c.tile([HID, 128], F32, tag="cps")
            nc.tensor.matmul(out=r32_ps[:, :], lhsT=ones_row[0:1, 0:HID],
                             rhs=recip[:, :])
            r32 = scratch.tile([HID, 128], F32, tag="r32")
            nc.scalar.copy(r32[:, :], r32_ps[:, :])
            g32_ps = ps_c.tile([HID, 128], F32, tag="cps")
            nc.tensor.matmul(out=g32_ps[:, :], lhsT=ones_row[0:1, 0:HID],
                             rhs=gn_row[:, :])
            g32 = scratch.tile([HID, 128], F32, tag="g32")
            nc.scalar.copy(g32[:, :], g32_ps[:, :])

            lT_ps = ps_c.tile([HID, 128], F32, tag="cps")
            nc.tensor.matmul(out=lT_ps[:, :], lhsT=UwT[:, :], rhs=mT[:, sl],
                             start=True, stop=False)
            nc.tensor.matmul(out=lT_ps[:, :], lhsT=Ub_row[:, :], rhs=s_row[:, :],
                             start=False, stop=True)

            lnorm = scratch.tile([HID, 128], F32, tag="lnorm")
            nc.vector.tensor_tensor(out=lnorm[:, :], in0=lT_ps[:, :],
                                    in1=r32[:, :], op=AF.mult)
            gpart = scratch.tile([HID, 128], F32, tag="gpart")
            nc.vector.tensor_tensor(out=gpart[:, :], in0=wg_sb[:, :],
                                    in1=g32[:, :], op=AF.mult)
            nc.scalar.activation(saT[0:HID, sl], gpart[:, :], RELU)
            nc.scalar.activation(saT[HID:2 * HID, sl], lnorm[:, :], RELU)

            for h, (w1T, w2T, w3T, b1c, b2c, b3c) in enumerate(head_sb):
                h1 = []
                for rh in range(2):
                    h_ps = ps_c.tile([128, 128], F32, tag="cps")
                    nc.tensor.matmul(out=h_ps[:, :],
                                     lhsT=w1T[:, rh * 128:(rh + 1) * 128],
                                     rhs=saT[:, sl])
                    h_sb = scratch.tile([128, 128], F32, tag="h1sb")
                    nc.scalar.activation(h_sb[:, :], h_ps[:, :], RELU,
                                         bias=b1c[:, rh:rh + 1])
                    h1.append(h_sb)
                h2 = []
                for rh in range(2):
                    h_ps = ps_c.tile([128, 128], F32, tag="cps")
                    for kh in range(2):
                        nc.tensor.matmul(out=h_ps[:, :],
                                         lhsT=w2T[kh][:, rh * 128:(rh + 1) * 128],
                                         rhs=h1[kh][:, :],
                                         start=(kh == 0), stop=(kh == 1))
                    h_sb = scratch.tile([128, 128], F32, tag="h2sb")
                    nc.scalar.activation(h_sb[:, :], h_ps[:, :], RELU,
                                         bias=b2c[:, rh:rh + 1])
                    h2.append(h_sb)
                q_ps = ps_c.tile([1, 128], F32, tag="cps")
                for kh in range(2):
                    nc.tensor.matmul(out=q_ps[:, :], lhsT=w3T[:, kh:kh + 1],
                                     rhs=h2[kh][:, :],
                                     start=(kh == 0), stop=(kh == 1))
                q_row = scratch.tile([1, 128], F32, tag="qrow")
                nc.scalar.activation(q_row[:, :], q_ps[:, :], IDENT,
                                     bias=b3c[:, :])
                nc.sync.dma_start(out_d[h:h + 1, sl], q_row[:, :])

        # ---------------- stream ----------------
        for ch in range(PF):
            issue_dma(ch)
        for ch in range(NCHUNK):
            if ch + PF < NCHUNK:
                issue_dma(ch + PF)
            compute_chunk(ch)
            if ch == TAIL_CH:
                phase_a_tail()
            if ch > CPB and (ch - 1) % CPB == 0:
                phase_c_block((ch - 1) // CPB - 1)
        phase_c_block(NB - 1)

    nc.compile()
    return nc


def _make_m16():
    m = np.zeros((128, SPC), np.float16)
    for p in range(128):
        m[p, p // PPS] = 1.0
    return m


def _make_esel():
    e = np.zeros((128, PPS * 128), np.float32)
    for r in range(PPS):
        for p in range(128):
            e[r * SPC + p // PPS, r * 128 + p] = 1.0
    return e


def _shard_inputs(inputs, b_loc=B_LOC):
    """Full inputs -> list of per-core in_maps."""
    m16 = _make_m16()
    esel = _make_esel()
    maps = []
    for c in range(NCORES):
        sl = slice(c * b_loc, (c + 1) * b_loc)
        m = {
            "local_states": np.ascontiguousarray(
                inputs["local_states"][sl].reshape(b_loc * L, L_DIM)),
            "global_states": np.ascontiguousarray(inputs["global_states"][sl]),
            "actions": np.ascontiguousarray(inputs["actions"][sl]),
            "m16": m16,
            "esel": esel,
        }
        for k in ("W_w", "W_b", "U_w", "U_b", "att_b",
                  "l1_w", "l1_b", "l2_w", "l2_b", "l3_w", "l3_b",
                  "l4_w", "l4_b", "l5_w", "l5_b", "l6_w", "l6_b"):
            m[k] = np.ascontiguousarray(np.asarray(inputs[k], np.float32))
        m["att_w"] = np.ascontiguousarray(
            np.asarray(inputs["att_w"], np.float32).reshape(1, 2 * HID))
        maps.append(m)
    return maps


_CACHE = {}


def kernel(**inputs) -> np.ndarray:
    from concourse.bass_utils import run_bass_kernel_spmd

    inputs = {k: np.asarray(v, np.float32) for k, v in inputs.items()}
    if "nc" not in _CACHE:
        _CACHE["nc"] = build_bass()
    nc = _CACHE["nc"]
    maps = _shard_inputs(inputs)
    res = run_bass_kernel_spmd(nc, maps, list(range(NCORES)))
    outs = [res.results[c]["out"] for c in range(NCORES)]  # each [2, B_LOC]
    q = np.concatenate(outs, axis=1)  # [2, B]
    return q.reshape(2, B, 1).astype(np.float32)


# revision 25
# speedup vs baseline: 1.0506x; 1.0506x over previous
"""Trainium2 Bass kernel for the Critic (gnn_message_passing) problem.

Math (per sample b):
  wg   = W_w @ g + W_b                                  [32]
  score_l = lrelu(x_l . v + c_b)   with v = U_w^T a2,
        c_b = a1.wg + att_b + U_b.a2
  score_g = lrelu((a1+a2).wg + att_b)
  total = score_g + sum_l score_l
  l_part = (U_w @ m_b + U_b * s_b) / total,  m_b = sum_l score_l x_l
  g_part = (score_g / total) * wg
  sa = [relu(g_part); relu(l_part); action]
  q_h = l3 @ relu(l2 @ relu(l1 @ sa + b1) + b2) + b3   (two heads)

Layout (per core, pure data parallel x8, B_LOC = 512 samples):
  - local_states streamed in 32 fp32 chunks [128 part, 25*128]; partition p
    holds 25 consecutive tokens of sample p//8. Chunk loads alternate
    between the two HWDGE rings (qSP / qAct), software-pipelined PF chunks
    ahead; setup loads are batched into few large DMAs so dispatch never
    congests the rings.
  - t = x.v via ONE custom DVE op per chunk: a scan(ADD, Src0*Src1)
    cumulative sum whose OUTPUT access pattern has inner stride 0 over
    each 128-elem segment, so only each segment's last value (the
    per-token dot product boundary) lands in a tiny [128, 26] tile.
    A second custom op computes score16 = fp16(lrelu(E[j+1]-E[j]+c)).
  - NO fp16 copy of x: the PE m-pass reads x as bf16 bitcast views
    (high halfword of each fp32 = truncated bf16), stride-2 APs.
    Validated: score fp16 + x bf16-trunc -> ~9.4e-3 final rel err.
  - scorem = score16 (x) m16 on DVE (GPSIMD stalls behind DVE's SBUF
    port during 2-src custom ops, so GPSIMD is kept out of the stream).
  - m accumulated on PE (25 matmuls of [128,16] into PSUM per chunk);
    s via one m16-stationary matmul + ACT accumulate.
  - Transposes for phase C are emitted mid-stream (after chunk 5); phase C
    (wg/sg + combine + head MLPs) runs per 128-sample block inside the
    stream, since a block's normalization only needs its own 8 chunks.
"""
import os
import sys

sys.path.insert(0, "/opt/trn_rl_repo")

from contextlib import ExitStack

import numpy as np

import concourse.bass as bass
import concourse.tile as tile
from concourse import bacc
from concourse import mybir
from concourse import dve_ops as DO
from concourse.dve_ops import TENSOR_TENSOR_REDUCE as CUSTOM_TTR
from concourse.dve_spec import (Spec, Src0, Src1, C0, C1, lower, AluOp, scan,
                                maxx, _has_src1)
from concourse.dve_uop import DveOpSpec

F32 = mybir.dt.float32
F16 = mybir.dt.float16
BF16 = mybir.dt.bfloat16
AF = mybir.AluOpType
IDENT = mybir.ActivationFunctionType.Identity
RELU = mybir.ActivationFunctionType.Relu

G_DIM, L_DIM, A_DIM, HID = 256, 128, 64, 32
B, L = 4096, 200
NCORES = 8
B_LOC = B // NCORES          # 512 samples per core
J = 25                       # tokens per partition per chunk
SPC = 16                     # samples per chunk (128 partitions / 8 per sample)
PPS = L // J                 # partitions per sample = 8
NCHUNK = B_LOC // SPC        # 32 chunks
NB = B_LOC // 128            # 128-sample blocks
CPB = NCHUNK // NB           # chunks per block = 8
NTOK = J * 128               # free-dim elements per chunk (3200)
PF = 7                       # chunk DMA prefetch distance
XBUFS = 10                   # x_ch ring depth (>= PF + 2)
TAIL_CH = 5                  # emit phase-A-tail transposes after this chunk


def _register_dve_op(name, spec, subdim=False):
    if name in DO._SUB_OPCODE_FOR_NAME:
        return next(op for op in DO.OPS if op.name == name)
    row = DO._CUSTOM_DVE_ROW_BASE + len(DO.OPS)
    assert row < 0x20
    DO._SUB_OPCODE_FOR_NAME[name] = row
    shas = {}
    for ver in ("v3", "v4"):
        shas[ver] = DveOpSpec(name=name, opcode=row, uops=lower(spec, ver=ver),
                              rd1_en=_has_src1(spec)).sha(ver)
    op = DO.DveOp(name, spec, subdim=subdim, uops_sha=shas)
    DO.OPS.append(op)
    DO.CUSTOM_DVE_SPECS[name] = spec
    return op


def _ref_cumsum_mul(in0, in1, c0, c1, c2):
    return np.cumsum(in0.astype(np.float32) * np.asarray(in1, np.float32),
                     axis=-1, dtype=np.float32)


def _ref_score(in0, in1, c0, c1, c2):
    dd = in0.astype(np.float32) - np.asarray(in1, np.float32) + c0
    return np.maximum(dd, dd * c1)


CUMSUM_MUL = _register_dve_op(
    "CUMSUM_MUL_ANT",
    Spec(body=scan(AluOp.ADD, Src0 * Src1), reference=_ref_cumsum_mul))

SCORE_OP = _register_dve_op(
    "DIFF_BIAS_LRELU_ANT",
    Spec(body=maxx((Src0 - Src1) + C0, ((Src0 - Src1) + C0) * C1),
         reference=_ref_score))


def build_bass(b_loc=B_LOC):
    tok = b_loc * L
    nc = bacc.Bacc()

    ls = nc.dram_tensor("local_states", [tok, L_DIM], F32, kind="ExternalInput")
    gs = nc.dram_tensor("global_states", [b_loc, G_DIM], F32, kind="ExternalInput")
    ac = nc.dram_tensor("actions", [b_loc, A_DIM], F32, kind="ExternalInput")
    Ww = nc.dram_tensor("W_w", [HID, G_DIM], F32, kind="ExternalInput")
    Wb = nc.dram_tensor("W_b", [HID], F32, kind="ExternalInput")
    Uw = nc.dram_tensor("U_w", [HID, L_DIM], F32, kind="ExternalInput")
    Ub = nc.dram_tensor("U_b", [HID], F32, kind="ExternalInput")
    attw = nc.dram_tensor("att_w", [1, 2 * HID], F32, kind="ExternalInput")
    attb = nc.dram_tensor("att_b", [1], F32, kind="ExternalInput")
    heads = []
    for h, names in enumerate((("l1", "l2", "l3"), ("l4", "l5", "l6"))):
        w1 = nc.dram_tensor(f"{names[0]}_w", [256, 128], F32, kind="ExternalInput")
        b1 = nc.dram_tensor(f"{names[0]}_b", [256], F32, kind="ExternalInput")
        w2 = nc.dram_tensor(f"{names[1]}_w", [256, 256], F32, kind="ExternalInput")
        b2 = nc.dram_tensor(f"{names[1]}_b", [256], F32, kind="ExternalInput")
        w3 = nc.dram_tensor(f"{names[2]}_w", [1, 256], F32, kind="ExternalInput")
        b3 = nc.dram_tensor(f"{names[2]}_b", [1], F32, kind="ExternalInput")
        heads.append((w1, b1, w2, b2, w3, b3))
    m16_d = nc.dram_tensor("m16", [128, SPC], F16, kind="ExternalInput")
    esel_d = nc.dram_tensor("esel", [128, PPS * 128], F32, kind="ExternalInput")
    out_d = nc.dram_tensor("out", [2, b_loc], F32, kind="ExternalOutput")

    with tile.TileContext(nc) as tc, ExitStack() as ctx:
        P = ctx.enter_context(tc.tile_pool(name="persist", bufs=1))
        scratch = ctx.enter_context(tc.tile_pool(name="scratch", bufs=2))
        ps_t = ctx.enter_context(tc.tile_pool(name="ps_t", bufs=2, space="PSUM"))

        # ---------------- Phase A: setup --------------------------------
        from concourse.masks import make_identity

        ident = P.tile([128, 128], F32, tag="ident")
        make_identity(nc, ident[:, :])
        ones_row = P.tile([1, 128], F32, tag="onesr")
        nc.vector.memset(ones_row[:, :], 1.0)

        # --- setup DMAs, sync ring (small weights + globals) ---
        m16_sb = P.tile([128, SPC], F16, tag="m16")
        nc.sync.dma_start(m16_sb[:, :], m16_d[:, :])
        esel = P.tile([128, PPS * 128], F32, tag="esel")
        nc.sync.dma_start(esel[:, :], esel_d[:, :])
        Ww_sb = P.tile([HID, G_DIM], F32, tag="Ww")
        nc.sync.dma_start(Ww_sb[:, :], Ww[:, :])
        Wb_sb = P.tile([HID, 1], F32, tag="Wb")
        nc.sync.dma_start(Wb_sb[:, :], Wb[:][:, None])
        Uw_sb = P.tile([HID, L_DIM], F32, tag="Uw")
        nc.sync.dma_start(Uw_sb[:, :], Uw[:, :])
        Ub_col = P.tile([HID, 1], F32, tag="Ubc")
        nc.sync.dma_start(Ub_col[:, :], Ub[:][:, None])
        Ub_row = P.tile([1, HID], F32, tag="Ubr")
        nc.sync.dma_start(Ub_row[:, :], Ub[:][None, :])
        a1_sb = P.tile([HID, 1], F32, tag="a1")
        nc.sync.dma_start(a1_sb[:, :], attw[0, 0:HID][:, None])
        a2_sb = P.tile([HID, 1], F32, tag="a2")
        nc.sync.dma_start(a2_sb[:, :], attw[0, HID:2 * HID][:, None])
        attb_sb = P.tile([1, 1], F32, tag="attb")
        nc.sync.dma_start(attb_sb[:, :], attb[:][None, :])
        # all 512 global states in one DMA: partition p <- sample bb*128+p
        g_all = P.tile([128, NB * G_DIM], F32, tag="gall")
        nc.sync.dma_start(
            g_all[:, :].rearrange("p (bb g) -> p bb g", g=G_DIM),
            gs[:, :].rearrange("(bb p) g -> p bb g", p=128))

        # --- setup DMAs, scalar ring (head weights + actions, batched) ---
        head_dma = []
        for hh, (w1, b1, w2, b2, w3, b3) in enumerate(heads):
            w1n = P.tile([128, 256], F32, tag=f"w1n{hh}")
            nc.scalar.dma_start(
                w1n[:, :].rearrange("p (r d) -> p r d", d=128),
                w1[:, :].rearrange("(r p) d -> p r d", p=128))
            w2n = P.tile([128, 512], F32, tag=f"w2n{hh}")
            nc.scalar.dma_start(
                w2n[:, :].rearrange("p (r d) -> p r d", d=256),
                w2[:, :].rearrange("(r p) d -> p r d", p=128))
            w3T = P.tile([128, 2], F32, tag=f"w3T{hh}")
            nc.scalar.dma_start(w3T[:, :],
                                w3[0, :].rearrange("(k p) -> p k", p=128))
            b1c = P.tile([128, 2], F32, tag=f"b1c{hh}")
            nc.scalar.dma_start(b1c[:, :],
                                b1[:].rearrange("(r p) -> p r", p=128))
            b2c = P.tile([128, 2], F32, tag=f"b2c{hh}")
            nc.scalar.dma_start(b2c[:, :],
                                b2[:].rearrange("(r p) -> p r", p=128))
            b3c = P.tile([1, 1], F32, tag=f"b3c{hh}")
            nc.scalar.dma_start(b3c[:, :], b3[:][None, :])
            head_dma.append((w1n, w2n, w3T, b1c, b2c, b3c))
        a_all = P.tile([128, NB * A_DIM], F32, tag="aall")
        nc.scalar.dma_start(
            a_all[:, :].rearrange("p (bb a) -> p bb a", a=A_DIM),
            ac[:, :].rearrange("(bb p) a -> p bb a", p=128))

        # --- v_flat [128, 3200] fp32 (PE + DVE copies only) ---
        v_ps = ps_t.tile([1, L_DIM], F32, tag="tps")
        nc.tensor.matmul(out=v_ps[:, :], lhsT=a2_sb[:, :], rhs=Uw_sb[:, :])
        v_row = P.tile([1, L_DIM], F32, tag="vrow")
        nc.vector.tensor_copy(v_row[:, :], v_ps[:, :])
        vrep_ps = ps_t.tile([128, 128], F32, tag="tps")
        nc.tensor.matmul(out=vrep_ps[:, :], lhsT=ones_row[:, :], rhs=v_row[:, :])
        v_flat = P.tile([128, NTOK], F32, tag="vflat")
        nc.vector.tensor_copy(v_flat[:, 0:128], vrep_ps[:, :])
        filled = 128
        while filled < NTOK:
            n = min(filled, NTOK - filled)
            nc.vector.tensor_copy(v_flat[:, filled:filled + n], v_flat[:, 0:n])
            filled += n

        # --- c_all [128, NCHUNK] ---
        u_ps = ps_t.tile([1, G_DIM], F32, tag="tps")
        nc.tensor.matmul(out=u_ps[:, :], lhsT=a1_sb[:, :], rhs=Ww_sb[:, :])
        u_row = P.tile([1, G_DIM], F32, tag="urow")
        nc.vector.tensor_copy(u_row[:, :], u_ps[:, :])
        urep_ps = ps_t.tile([128, G_DIM], F32, tag="tps")
        nc.tensor.matmul(out=urep_ps[:, :], lhsT=ones_row[:, :], rhs=u_row[:, :])
        u_rep = P.tile([128, G_DIM], F32, tag="urep")
        nc.vector.tensor_copy(u_rep[:, :], urep_ps[:, :])
        # c_col4 off the DVE: multiply on GPSIMD, reduce via ACT accumulate
        # (both engines idle pre-stream; shortens the DVE queue before cum0)
        c_col4 = P.tile([128, NB], F32, tag="ccol4")
        for bb in range(NB):
            junkA = scratch.tile([128, G_DIM], F32, tag="junkA",
                                 name=f"junkA{bb}")
            nc.gpsimd.tensor_tensor(
                out=junkA[:, :],
                in0=g_all[:, bb * G_DIM:(bb + 1) * G_DIM],
                in1=u_rep[:, :], op=AF.mult)
            junkB = scratch.tile([128, G_DIM], F32, tag="junkB",
                                 name=f"junkB{bb}")
            nc.scalar.activation(junkB[:, :], junkA[:, :], IDENT,
                                 accum_out=c_col4[:, bb:bb + 1])
        uba2_ps = ps_t.tile([1, 1], F32, tag="tps")
        nc.tensor.matmul(out=uba2_ps[:, :], lhsT=Ub_col[:, :], rhs=a2_sb[:, :],
                         start=True, stop=False, skip_group_check=True)
        nc.tensor.matmul(out=uba2_ps[:, :], lhsT=Wb_sb[:, :], rhs=a1_sb[:, :],
                         start=False, stop=True, skip_group_check=True)
        cconst = P.tile([1, 1], F32, tag="cconst")
        nc.vector.tensor_tensor(out=cconst[:, :], in0=uba2_ps[:, :],
                                in1=attb_sb[:, :], op=AF.add)
        cc128_ps = ps_t.tile([128, 1], F32, tag="tps")
        nc.tensor.matmul(out=cc128_ps[:, :], lhsT=ones_row[0:1, :],
                         rhs=cconst[:, :])
        cc128 = P.tile([128, 1], F32, tag="cc128")
        nc.vector.tensor_copy(cc128[:, :], cc128_ps[:, :])
        call_ps = ps_t.tile([128, NCHUNK], F32, tag="tps")
        for r in range(PPS):
            nc.tensor.matmul(out=call_ps[:, r:NCHUNK:PPS],
                             lhsT=esel[:, r * 128:(r + 1) * 128],
                             rhs=c_col4[:, :], skip_group_check=True)
        c_all = P.tile([128, NCHUNK], F32, tag="call")
        nc.scalar.activation(c_all[:, :], call_ps[:, :], IDENT,
                             bias=cc128[:, :])

        a12 = P.tile([HID, 1], F32, tag="a12")
        nc.vector.tensor_tensor(out=a12[:, :], in0=a1_sb[:, :], in1=a2_sb[:, :],
                                op=AF.add)

        # ---------------- pools for stream + phase C ----------------
        xpool = ctx.enter_context(tc.tile_pool(name="xchunk", bufs=XBUFS))
        cumpool = ctx.enter_context(tc.tile_pool(name="cump", bufs=3))
        scpool = ctx.enter_context(tc.tile_pool(name="score", bufs=3))
        smpool = ctx.enter_context(tc.tile_pool(name="scorem", bufs=2))
        jpool = ctx.enter_context(tc.tile_pool(name="junk", bufs=2))
        ps_m = ctx.enter_context(tc.tile_pool(name="ps_m", bufs=2, space="PSUM"))
        ps_s = ctx.enter_context(tc.tile_pool(name="ps_s", bufs=1, space="PSUM"))
        ps_c = ctx.enter_context(tc.tile_pool(name="ps_c", bufs=3, space="PSUM"))

        mT = P.tile([L_DIM, b_loc], F32, tag="mT")
        s_colT = P.tile([SPC, NCHUNK], F32, tag="scolT")
        saT = P.tile([128, b_loc], F32, tag="saT")

        # filled by the phase-A tail (emitted mid-stream)
        gT = [P.tile([128, b_loc], F32, tag=f"gT{g}", name=f"gT{g}")
              for g in range(G_DIM // 128)]
        WwT = [P.tile([128, HID], F32, tag=f"WwT{g}", name=f"WwT{g}")
               for g in range(G_DIM // 128)]
        UwT = P.tile([L_DIM, HID], F32, tag="UwT")
        head_sb = []

        x_tiles = [None] * NCHUNK

        def transpose_to_sbuf(dst_ap, src_ap):
            pp, ff = src_ap.shape
            t_ps = ps_t.tile([128, 128], F32, tag="tps")
            nc.tensor.transpose(t_ps[0:ff, 0:pp], src_ap, ident[0:pp, 0:pp])
            nc.scalar.copy(dst_ap, t_ps[0:ff, 0:pp])

        def issue_dma(ch):
            x_ch = xpool.tile([128, NTOK], F32, tag="xch")
            src = ls[ch * NTOK:(ch + 1) * NTOK, :]
            dma_eng = nc.sync if ch % 2 == 0 else nc.scalar
            dma_eng.dma_start(
                x_ch[:, :], src.rearrange("(p j) d -> p (j d)", p=128))
            x_tiles[ch] = x_ch

        def compute_chunk(ch):
            x_ch = x_tiles[ch]
            # boundary-only cumsum(x * v): inner write stride 0, so each
            # 128-segment's final running sum lands at cum[:, 1+j].
            cum = cumpool.tile([128, 32], F32, tag="cum")
            nc.gpsimd.memset(cum[:, 0:1], 0.0)
            nc.vector._custom_dve(
                CUMSUM_MUL,
                out=cum[:, 1:J + 1, None].broadcast_to((128, J, 128)),
                in0=x_ch[:, :].rearrange("p (j d) -> p j d", d=128),
                in1=v_flat[:, :].rearrange("p (j d) -> p j d", d=128))

            # score16[p, j] = fp16(lrelu(E[j+1] - E[j] + c))
            score16 = scpool.tile([128, J], F16, tag="sc")
            nc.vector._custom_dve(SCORE_OP, out=score16[:, :],
                                  in0=cum[:, 1:J + 1],
                                  in1=cum[:, 0:J],
                                  s0=c_all[:, ch:ch + 1], s1=0.01)

            # scorem[p, (j,s)] = score16[p,j] * m16[p,s]
            scorem = smpool.tile([128, J * SPC], F16, tag="sm")
            sm3 = scorem[:, :].rearrange("p (j s) -> p j s", s=SPC)
            nc.vector.tensor_tensor(
                out=sm3[:, :, :],
                in0=score16[:, :, None].broadcast_to((128, J, SPC)),
                in1=m16_sb[:, None, :].broadcast_to((128, J, SPC)),
                op=AF.mult)

            # mT_chunk [128 feat, 16] accumulated over the 25 j-tiles;
            # lhsT is the bf16 high-halfword view of the fp32 x tile.
            mT_ps = ps_m.tile([L_DIM, SPC], F32, tag="mps")
            for j in range(J):
                xbj = x_ch[:, j * 128:(j + 1) * 128].bitcast(BF16)[:, 1:256:2]
                nc.tensor.matmul(out=mT_ps[:, :],
                                 lhsT=xbj,
                                 rhs=scorem[:, j * SPC:(j + 1) * SPC],
                                 start=(j == 0), stop=(j == J - 1))
            nc.scalar.copy(mT[:, ch * SPC:(ch + 1) * SPC], mT_ps[:, :])

            # s[s] = sum_{p,j} score16[p,j]*ind(p//8==s)  -> s_colT[:, ch]
            s_ps = ps_s.tile([SPC, J], F32, tag="sps")
            nc.tensor.matmul(out=s_ps[:, :], lhsT=m16_sb[:, :],
                             rhs=score16[:, :])
            junk_s = jpool.tile([SPC, J], F32, tag="jks")
            nc.scalar.activation(junk_s[:, :], s_ps[:, :], IDENT,
                                 accum_out=s_colT[:, ch:ch + 1])

        def phase_a_tail():
            for bb in range(NB):
                for g in range(G_DIM // 128):
                    transpose_to_sbuf(
                        gT[g][:, bb * 128:(bb + 1) * 128],
                        g_all[:, bb * G_DIM + g * 128:bb * G_DIM + (g + 1) * 128])
            for g in range(G_DIM // 128):
                transpose_to_sbuf(WwT[g][:, :], Ww_sb[:, g * 128:(g + 1) * 128])
            transpose_to_sbuf(UwT[:, :], Uw_sb[:, :])
            for bb in range(NB):
                transpose_to_sbuf(
                    saT[2 * HID:2 * HID + A_DIM, bb * 128:(bb + 1) * 128],
                    a_all[:, bb * A_DIM:(bb + 1) * A_DIM])
            for hh, (w1n, w2n, w3T, b1c, b2c, b3c) in enumerate(head_dma):
                w1T = P.tile([128, 256], F32, tag=f"w1T{hh}")
                for rh in range(2):
                    transpose_to_sbuf(w1T[:, rh * 128:(rh + 1) * 128],
                                      w1n[:, rh * 128:(rh + 1) * 128])
                w2T = [P.tile([128, 256], F32, tag=f"w2T{hh}_{kh}",
                              name=f"w2T{hh}_{kh}") for kh in range(2)]
                for rh in range(2):
                    for kh in range(2):
                        transpose_to_sbuf(
                            w2T[kh][:, rh * 128:(rh + 1) * 128],
                            w2n[:, rh * 256 + kh * 128:rh * 256 + (kh + 1) * 128])
                head_sb.append((w1T, w2T, w3T, b1c, b2c, b3c))

        def phase_c_block(bb):
            sl = slice(bb * 128, (bb + 1) * 128)
            ch0 = bb * CPB
            # wg block [32, 128] + sg row
            wg_ps = ps_c.tile([HID, 128], F32, tag="cps")
            for g in range(G_DIM // 128):
                nc.tensor.matmul(out=wg_ps[:, :], lhsT=WwT[g][:, :],
                                 rhs=gT[g][:, sl],
                                 start=(g == 0), stop=(g == G_DIM // 128 - 1))
            wg_sb = scratch.tile([HID, 128], F32, tag="wgsb")
            nc.scalar.activation(wg_sb[:, :], wg_ps[:, :], IDENT,
                                 bias=Wb_sb[:, :])
            sg_ps = ps_c.tile([1, 128], F32, tag="cps")
            nc.tensor.matmul(out=sg_ps[:, :], lhsT=a12[:, :], rhs=wg_sb[:, :])
            sg_lin = scratch.tile([1, 128], F32, tag="sglin")
            nc.scalar.activation(sg_lin[:, :], sg_ps[:, :], IDENT,
                                 bias=attb_sb[:, :])
            sg_raw = scratch.tile([1, 128], F32, tag="sgraw")
            nc.vector.scalar_tensor_tensor(out=sg_raw[:, :], in0=sg_lin[:, :],
                                           scalar=0.01, in1=sg_lin[:, :],
                                           op0=AF.mult, op1=AF.max)

            # s_row block [1, 128]: sample ch*16+s -> col (ch-ch0)*16+s
            srow_ps = ps_c.tile([1, 128], F32, tag="cps")
            for s in range(SPC):
                nc.tensor.matmul(out=srow_ps[0:1, s * CPB:(s + 1) * CPB],
                                 lhsT=ident[0:SPC, s:s + 1],
                                 rhs=s_colT[:, ch0:ch0 + CPB],
                                 skip_group_check=True)
            s_row = scratch.tile([1, 128], F32, tag="srow")
            nc.scalar.copy(
                s_row[0:1, :].rearrange("one (c s) -> one c s", s=SPC),
                srow_ps[0:1, :].rearrange("one (s c) -> one c s", s=SPC))

            total = scratch.tile([1, 128], F32, tag="total")
            nc.vector.tensor_tensor(out=total[:, :], in0=sg_raw[:, :],
                                    in1=s_row[:, :], op=AF.add)
            recip = scratch.tile([1, 128], F32, tag="recip")
            nc.vector.reciprocal_approx_fast(recip[:, :], total[:, :])
            gn_row = scratch.tile([1, 128], F32, tag="gn")
            nc.vector.tensor_tensor(out=gn_row[:, :], in0=sg_raw[:, :],
                                    in1=recip[:, :], op=AF.mult)

            r32_ps = ps_c.tile([HID, 128], F32, tag="cps")
            nc.tensor.matmul(out=r32_ps[:, :], lhsT=ones_row[0:1, 0:HID],
                             rhs=recip[:, :])
            r32 = scratch.tile([HID, 128], F32, tag="r32")
            nc.scalar.copy(r32[:, :], r32_ps[:, :])
            g32_ps = ps_c.tile([HID, 128], F32, tag="cps")
            nc.tensor.matmul(out=g32_ps[:, :], lhsT=ones_row[0:1, 0:HID],
                             rhs=gn_row[:, :])
            g32 = scratch.tile([HID, 128], F32, tag="g32")
            nc.scalar.copy(g32[:, :], g32_ps[:, :])

            lT_ps = ps_c.tile([HID, 128], F32, tag="cps")
            nc.tensor.matmul(out=lT_ps[:, :], lhsT=UwT[:, :], rhs=mT[:, sl],
                             start=True, stop=False)
            nc.tensor.matmul(out=lT_ps[:, :], lhsT=Ub_row[:, :], rhs=s_row[:, :],
                             start=False, stop=True)

            lnorm = scratch.tile([HID, 128], F32, tag="lnorm")
            nc.vector.tensor_tensor(out=lnorm[:, :], in0=lT_ps[:, :],
                                    in1=r32[:, :], op=AF.mult)
            gpart = scratch.tile([HID, 128], F32, tag="gpart")
            nc.vector.tensor_tensor(out=gpart[:, :], in0=wg_sb[:, :],
                                    in1=g32[:, :], op=AF.mult)
            nc.scalar.activation(saT[0:HID, sl], gpart[:, :], RELU)
            nc.scalar.activation(saT[HID:2 * HID, sl], lnorm[:, :], RELU)

            for h, (w1T, w2T, w3T, b1c, b2c, b3c) in enumerate(head_sb):
                h1 = []
                for rh in range(2):
                    h_ps = ps_c.tile([128, 128], F32, tag="cps")
                    nc.tensor.matmul(out=h_ps[:, :],
                                     lhsT=w1T[:, rh * 128:(rh + 1) * 128],
                                     rhs=saT[:, sl])
                    h_sb = scratch.tile([128, 128], F32, tag="h1sb")
                    nc.scalar.activation(h_sb[:, :], h_ps[:, :], RELU,
                                         bias=b1c[:, rh:rh + 1])
                    h1.append(h_sb)
                h2 = []
                for rh in range(2):
                    h_ps = ps_c.tile([128, 128], F32, tag="cps")
                    for kh in range(2):
                        nc.tensor.matmul(out=h_ps[:, :],
                                         lhsT=w2T[kh][:, rh * 128:(rh + 1) * 128],
                                         rhs=h1[kh][:, :],
                                         start=(kh == 0), stop=(kh == 1))
                    h_sb = scratch.tile([128, 128], F32, tag="h2sb")
                    nc.scalar.activation(h_sb[:, :], h_ps[:, :], RELU,
                                         bias=b2c[:, rh:rh + 1])
                    h2.append(h_sb)
                q_ps = ps_c.tile([1, 128], F32, tag="cps")
                for kh in range(2):
                    nc.tensor.matmul(out=q_ps[:, :], lhsT=w3T[:, kh:kh + 1],
                                     rhs=h2[kh][:, :],
                                     start=(kh == 0), stop=(kh == 1))
                q_row = scratch.tile([1, 128], F32, tag="qrow")
                nc.scalar.activation(q_row[:, :], q_ps[:, :], IDENT,
                                     bias=b3c[:, :])
                nc.sync.dma_start(out_d[h:h + 1, sl], q_row[:, :])

        # ---------------- stream ----------------
        for ch in range(PF):
            issue_dma(ch)
        for ch in range(NCHUNK):
            if ch + PF < NCHUNK:
                issue_dma(ch + PF)
            compute_chunk(ch)
            if ch == TAIL_CH:
                phase_a_tail()
            if ch > CPB and (ch - 1) % CPB == 0:
                phase_c_block((ch - 1) // CPB - 1)
        phase_c_block(NB - 1)

    nc.compile()
    return nc


def _make_m16():
    m = np.zeros((128, SPC), np.float16)
    for p in range(128):
        m[p, p // PPS] = 1.0
    return m


def _make_esel():
    e = np.zeros((128, PPS * 128), np.float32)
    for r in range(PPS):
        for p in range(128):
            e[r * SPC + p // PPS, r * 128 + p] = 1.0
    return e


def _shard_inputs(inputs, b_loc=B_LOC):
    """Full inputs -> list of per-core in_maps."""
    m16 = _make_m16()
    esel = _make_esel()
    maps = []
    for c in range(NCORES):
        sl = slice(c * b_loc, (c + 1) * b_loc)
        m = {
            "local_states": np.ascontiguousarray(
                inputs["local_states"][sl].reshape(b_loc * L, L_DIM)),
            "global_states": np.ascontiguousarray(inputs["global_states"][sl]),
            "actions": np.ascontiguousarray(inputs["actions"][sl]),
            "m16": m16,
            "esel": esel,
        }
        for k in ("W_w", "W_b", "U_w", "U_b", "att_b",
                  "l1_w", "l1_b", "l2_w", "l2_b", "l3_w", "l3_b",
                  "l4_w", "l4_b", "l5_w", "l5_b", "l6_w", "l6_b"):
            m[k] = np.ascontiguousarray(np.asarray(inputs[k], np.float32))
        m["att_w"] = np.ascontiguousarray(
            np.asarray(inputs["att_w"], np.float32).reshape(1, 2 * HID))
        maps.append(m)
    return maps


_CACHE = {}


def kernel(**inputs) -> np.ndarray:
    from concourse.bass_utils import run_bass_kernel_spmd

    inputs = {k: np.asarray(v, np.float32) for k, v in inputs.items()}
    if "nc" not in _CACHE:
        _CACHE["nc"] = build_bass()
    nc = _CACHE["nc"]
    maps = _shard_inputs(inputs)
    res = run_bass_kernel_spmd(nc, maps, list(range(NCORES)))
    outs = [res.results[c]["out"] for c in range(NCORES)]  # each [2, B_LOC]
    q = np.concatenate(outs, axis=1)  # [2, B]
    return q.reshape(2, B, 1).astype(np.float32)


# revision 26
# speedup vs baseline: 1.0837x; 1.0314x over previous
"""Trainium2 Bass kernel for the Critic (gnn_message_passing) problem.

Math (per sample b):
  wg   = W_w @ g + W_b                                  [32]
  score_l = lrelu(x_l . v + c_b)   with v = U_w^T a2,
        c_b = a1.wg + att_b + U_b.a2
  score_g = lrelu((a1+a2).wg + att_b)
  total = score_g + sum_l score_l
  l_part = (U_w @ m_b + U_b * s_b) / total,  m_b = sum_l score_l x_l
  g_part = (score_g / total) * wg
  sa = [relu(g_part); relu(l_part); action]
  q_h = l3 @ relu(l2 @ relu(l1 @ sa + b1) + b2) + b3   (two heads)

Layout (per core, pure data parallel x8, B_LOC = 512 samples):
  - local_states streamed in 32 fp32 chunks [128 part, 25*128]; partition p
    holds 25 consecutive tokens of sample p//8. Chunk loads alternate
    between the two HWDGE rings (qSP / qAct), software-pipelined PF chunks
    ahead; setup loads are batched into few large DMAs so dispatch never
    congests the rings.
  - t = x.v via ONE custom DVE op per chunk: a scan(ADD, Src0*Src1)
    cumulative sum whose OUTPUT access pattern has inner stride 0 over
    each 128-elem segment, so only each segment's last value (the
    per-token dot product boundary) lands in a tiny [128, 26] tile.
    A second custom op computes score16 = fp16(lrelu(E[j+1]-E[j]+c)).
  - NO fp16 copy of x: the PE m-pass reads x as bf16 bitcast views
    (high halfword of each fp32 = truncated bf16), stride-2 APs.
    Validated: score fp16 + x bf16-trunc -> ~9.4e-3 final rel err.
  - scorem = score16 (x) m16 on DVE (GPSIMD stalls behind DVE's SBUF
    port during 2-src custom ops, so GPSIMD is kept out of the stream).
  - m accumulated on PE (25 matmuls of [128,16] into PSUM per chunk);
    s via one m16-stationary matmul + ACT accumulate.
  - Transposes for phase C are emitted mid-stream (after chunk 5); phase C
    (wg/sg + combine + head MLPs) runs per 128-sample block inside the
    stream, since a block's normalization only needs its own 8 chunks.
"""
import os
import sys

sys.path.insert(0, "/opt/trn_rl_repo")

from contextlib import ExitStack

import numpy as np

import concourse.bass as bass
import concourse.tile as tile
from concourse import bacc
from concourse import mybir
from concourse import dve_ops as DO
from concourse.dve_ops import TENSOR_TENSOR_REDUCE as CUSTOM_TTR
from concourse.dve_spec import (Spec, Src0, Src1, C0, C1, lower, AluOp, scan,
                                maxx, _has_src1)
from concourse.dve_uop import DveOpSpec

F32 = mybir.dt.float32
F16 = mybir.dt.float16
BF16 = mybir.dt.bfloat16
AF = mybir.AluOpType
IDENT = mybir.ActivationFunctionType.Identity
RELU = mybir.ActivationFunctionType.Relu

G_DIM, L_DIM, A_DIM, HID = 256, 128, 64, 32
B, L = 4096, 200
NCORES = 8
B_LOC = B // NCORES          # 512 samples per core
J = 25                       # tokens per partition per chunk
SPC = 16                     # samples per chunk (128 partitions / 8 per sample)
PPS = L // J                 # partitions per sample = 8
NCHUNK = B_LOC // SPC        # 32 chunks
NB = B_LOC // 128            # 128-sample blocks
CPB = NCHUNK // NB           # chunks per block = 8
NTOK = J * 128               # free-dim elements per chunk (3200)
PF = 7                       # chunk DMA prefetch distance
XBUFS = 10                   # x_ch ring depth (>= PF + 2)
TAIL_CH = 3                  # emit phase-A-tail transposes after this chunk


def _register_dve_op(name, spec, subdim=False):
    if name in DO._SUB_OPCODE_FOR_NAME:
        return next(op for op in DO.OPS if op.name == name)
    row = DO._CUSTOM_DVE_ROW_BASE + len(DO.OPS)
    assert row < 0x20
    DO._SUB_OPCODE_FOR_NAME[name] = row
    shas = {}
    for ver in ("v3", "v4"):
        shas[ver] = DveOpSpec(name=name, opcode=row, uops=lower(spec, ver=ver),
                              rd1_en=_has_src1(spec)).sha(ver)
    op = DO.DveOp(name, spec, subdim=subdim, uops_sha=shas)
    DO.OPS.append(op)
    DO.CUSTOM_DVE_SPECS[name] = spec
    return op


def _ref_cumsum_mul(in0, in1, c0, c1, c2):
    return np.cumsum(in0.astype(np.float32) * np.asarray(in1, np.float32),
                     axis=-1, dtype=np.float32)


def _ref_score(in0, in1, c0, c1, c2):
    dd = in0.astype(np.float32) - np.asarray(in1, np.float32) + c0
    return np.maximum(dd, dd * c1)


CUMSUM_MUL = _register_dve_op(
    "CUMSUM_MUL_ANT",
    Spec(body=scan(AluOp.ADD, Src0 * Src1), reference=_ref_cumsum_mul))

SCORE_OP = _register_dve_op(
    "DIFF_BIAS_LRELU_ANT",
    Spec(body=maxx((Src0 - Src1) + C0, ((Src0 - Src1) + C0) * C1),
         reference=_ref_score))


def build_bass(b_loc=B_LOC):
    tok = b_loc * L
    nc = bacc.Bacc()

    ls = nc.dram_tensor("local_states", [tok, L_DIM], F32, kind="ExternalInput")
    gs = nc.dram_tensor("global_states", [b_loc, G_DIM], F32, kind="ExternalInput")
    ac = nc.dram_tensor("actions", [b_loc, A_DIM], F32, kind="ExternalInput")
    Ww = nc.dram_tensor("W_w", [HID, G_DIM], F32, kind="ExternalInput")
    Wb = nc.dram_tensor("W_b", [HID], F32, kind="ExternalInput")
    Uw = nc.dram_tensor("U_w", [HID, L_DIM], F32, kind="ExternalInput")
    Ub = nc.dram_tensor("U_b", [HID], F32, kind="ExternalInput")
    attw = nc.dram_tensor("att_w", [1, 2 * HID], F32, kind="ExternalInput")
    attb = nc.dram_tensor("att_b", [1], F32, kind="ExternalInput")
    heads = []
    for h, names in enumerate((("l1", "l2", "l3"), ("l4", "l5", "l6"))):
        w1 = nc.dram_tensor(f"{names[0]}_w", [256, 128], F32, kind="ExternalInput")
        b1 = nc.dram_tensor(f"{names[0]}_b", [256], F32, kind="ExternalInput")
        w2 = nc.dram_tensor(f"{names[1]}_w", [256, 256], F32, kind="ExternalInput")
        b2 = nc.dram_tensor(f"{names[1]}_b", [256], F32, kind="ExternalInput")
        w3 = nc.dram_tensor(f"{names[2]}_w", [1, 256], F32, kind="ExternalInput")
        b3 = nc.dram_tensor(f"{names[2]}_b", [1], F32, kind="ExternalInput")
        heads.append((w1, b1, w2, b2, w3, b3))
    m16_d = nc.dram_tensor("m16", [128, SPC], F16, kind="ExternalInput")
    esel_d = nc.dram_tensor("esel", [128, PPS * 128], F32, kind="ExternalInput")
    out_d = nc.dram_tensor("out", [2, b_loc], F32, kind="ExternalOutput")

    with tile.TileContext(nc) as tc, ExitStack() as ctx:
        P = ctx.enter_context(tc.tile_pool(name="persist", bufs=1))
        scratch = ctx.enter_context(tc.tile_pool(name="scratch", bufs=2))
        ps_t = ctx.enter_context(tc.tile_pool(name="ps_t", bufs=2, space="PSUM"))

        # ---------------- Phase A: setup --------------------------------
        from concourse.masks import make_identity

        ident = P.tile([128, 128], F32, tag="ident")
        make_identity(nc, ident[:, :])
        ones_row = P.tile([1, 128], F32, tag="onesr")
        nc.vector.memset(ones_row[:, :], 1.0)

        # --- setup DMAs, sync ring (small weights + globals) ---
        m16_sb = P.tile([128, SPC], F16, tag="m16")
        nc.sync.dma_start(m16_sb[:, :], m16_d[:, :])
        esel = P.tile([128, PPS * 128], F32, tag="esel")
        nc.sync.dma_start(esel[:, :], esel_d[:, :])
        Ww_sb = P.tile([HID, G_DIM], F32, tag="Ww")
        nc.sync.dma_start(Ww_sb[:, :], Ww[:, :])
        Wb_sb = P.tile([HID, 1], F32, tag="Wb")
        nc.sync.dma_start(Wb_sb[:, :], Wb[:][:, None])
        Uw_sb = P.tile([HID, L_DIM], F32, tag="Uw")
        nc.sync.dma_start(Uw_sb[:, :], Uw[:, :])
        Ub_col = P.tile([HID, 1], F32, tag="Ubc")
        nc.sync.dma_start(Ub_col[:, :], Ub[:][:, None])
        Ub_row = P.tile([1, HID], F32, tag="Ubr")
        nc.sync.dma_start(Ub_row[:, :], Ub[:][None, :])
        a1_sb = P.tile([HID, 1], F32, tag="a1")
        nc.sync.dma_start(a1_sb[:, :], attw[0, 0:HID][:, None])
        a2_sb = P.tile([HID, 1], F32, tag="a2")
        nc.sync.dma_start(a2_sb[:, :], attw[0, HID:2 * HID][:, None])
        attb_sb = P.tile([1, 1], F32, tag="attb")
        nc.sync.dma_start(attb_sb[:, :], attb[:][None, :])
        # all 512 global states in one DMA: partition p <- sample bb*128+p
        g_all = P.tile([128, NB * G_DIM], F32, tag="gall")
        nc.sync.dma_start(
            g_all[:, :].rearrange("p (bb g) -> p bb g", g=G_DIM),
            gs[:, :].rearrange("(bb p) g -> p bb g", p=128))

        # --- setup DMAs, scalar ring (head weights + actions, batched) ---
        head_dma = []
        for hh, (w1, b1, w2, b2, w3, b3) in enumerate(heads):
            w1n = P.tile([128, 256], F32, tag=f"w1n{hh}")
            nc.scalar.dma_start(
                w1n[:, :].rearrange("p (r d) -> p r d", d=128),
                w1[:, :].rearrange("(r p) d -> p r d", p=128))
            w2n = P.tile([128, 512], F32, tag=f"w2n{hh}")
            nc.scalar.dma_start(
                w2n[:, :].rearrange("p (r d) -> p r d", d=256),
                w2[:, :].rearrange("(r p) d -> p r d", p=128))
            w3T = P.tile([128, 2], F32, tag=f"w3T{hh}")
            nc.scalar.dma_start(w3T[:, :],
                                w3[0, :].rearrange("(k p) -> p k", p=128))
            b1c = P.tile([128, 2], F32, tag=f"b1c{hh}")
            nc.scalar.dma_start(b1c[:, :],
                                b1[:].rearrange("(r p) -> p r", p=128))
            b2c = P.tile([128, 2], F32, tag=f"b2c{hh}")
            nc.scalar.dma_start(b2c[:, :],
                                b2[:].rearrange("(r p) -> p r", p=128))
            b3c = P.tile([1, 1], F32, tag=f"b3c{hh}")
            nc.scalar.dma_start(b3c[:, :], b3[:][None, :])
            head_dma.append((w1n, w2n, w3T, b1c, b2c, b3c))
        a_all = P.tile([128, NB * A_DIM], F32, tag="aall")
        nc.scalar.dma_start(
            a_all[:, :].rearrange("p (bb a) -> p bb a", a=A_DIM),
            ac[:, :].rearrange("(bb p) a -> p bb a", p=128))

        # --- v_flat [128, 3200] fp32 (PE + DVE copies only) ---
        v_ps = ps_t.tile([1, L_DIM], F32, tag="tps")
        nc.tensor.matmul(out=v_ps[:, :], lhsT=a2_sb[:, :], rhs=Uw_sb[:, :])
        v_row = P.tile([1, L_DIM], F32, tag="vrow")
        nc.vector.tensor_copy(v_row[:, :], v_ps[:, :])
        vrep_ps = ps_t.tile([128, 128], F32, tag="tps")
        nc.tensor.matmul(out=vrep_ps[:, :], lhsT=ones_row[:, :], rhs=v_row[:, :])
        v_flat = P.tile([128, NTOK], F32, tag="vflat")
        nc.vector.tensor_copy(v_flat[:, 0:128], vrep_ps[:, :])
        filled = 128
        while filled < NTOK:
            n = min(filled, NTOK - filled)
            nc.vector.tensor_copy(v_flat[:, filled:filled + n], v_flat[:, 0:n])
            filled += n

        # --- c_all [128, NCHUNK] ---
        u_ps = ps_t.tile([1, G_DIM], F32, tag="tps")
        nc.tensor.matmul(out=u_ps[:, :], lhsT=a1_sb[:, :], rhs=Ww_sb[:, :])
        u_row = P.tile([1, G_DIM], F32, tag="urow")
        nc.vector.tensor_copy(u_row[:, :], u_ps[:, :])
        urep_ps = ps_t.tile([128, G_DIM], F32, tag="tps")
        nc.tensor.matmul(out=urep_ps[:, :], lhsT=ones_row[:, :], rhs=u_row[:, :])
        u_rep = P.tile([128, G_DIM], F32, tag="urep")
        nc.vector.tensor_copy(u_rep[:, :], urep_ps[:, :])
        # c_col4 off the DVE: multiply on GPSIMD, reduce via ACT accumulate
        # (both engines idle pre-stream; shortens the DVE queue before cum0)
        c_col4 = P.tile([128, NB], F32, tag="ccol4")
        for bb in range(NB):
            junkA = scratch.tile([128, G_DIM], F32, tag="junkA",
                                 name=f"junkA{bb}")
            nc.gpsimd.tensor_tensor(
                out=junkA[:, :],
                in0=g_all[:, bb * G_DIM:(bb + 1) * G_DIM],
                in1=u_rep[:, :], op=AF.mult)
            junkB = scratch.tile([128, G_DIM], F32, tag="junkB",
                                 name=f"junkB{bb}")
            nc.scalar.activation(junkB[:, :], junkA[:, :], IDENT,
                                 accum_out=c_col4[:, bb:bb + 1])
        uba2_ps = ps_t.tile([1, 1], F32, tag="tps")
        nc.tensor.matmul(out=uba2_ps[:, :], lhsT=Ub_col[:, :], rhs=a2_sb[:, :],
                         start=True, stop=False, skip_group_check=True)
        nc.tensor.matmul(out=uba2_ps[:, :], lhsT=Wb_sb[:, :], rhs=a1_sb[:, :],
                         start=False, stop=True, skip_group_check=True)
        cconst = P.tile([1, 1], F32, tag="cconst")
        nc.vector.tensor_tensor(out=cconst[:, :], in0=uba2_ps[:, :],
                                in1=attb_sb[:, :], op=AF.add)
        cc128_ps = ps_t.tile([128, 1], F32, tag="tps")
        nc.tensor.matmul(out=cc128_ps[:, :], lhsT=ones_row[0:1, :],
                         rhs=cconst[:, :])
        cc128 = P.tile([128, 1], F32, tag="cc128")
        nc.vector.tensor_copy(cc128[:, :], cc128_ps[:, :])
        call_ps = ps_t.tile([128, NCHUNK], F32, tag="tps")
        for r in range(PPS):
            nc.tensor.matmul(out=call_ps[:, r:NCHUNK:PPS],
                             lhsT=esel[:, r * 128:(r + 1) * 128],
                             rhs=c_col4[:, :], skip_group_check=True)
        c_all = P.tile([128, NCHUNK], F32, tag="call")
        nc.scalar.activation(c_all[:, :], call_ps[:, :], IDENT,
                             bias=cc128[:, :])

        a12 = P.tile([HID, 1], F32, tag="a12")
        nc.vector.tensor_tensor(out=a12[:, :], in0=a1_sb[:, :], in1=a2_sb[:, :],
                                op=AF.add)

        # ---------------- pools for stream + phase C ----------------
        xpool = ctx.enter_context(tc.tile_pool(name="xchunk", bufs=XBUFS))
        cumpool = ctx.enter_context(tc.tile_pool(name="cump", bufs=3))
        scpool = ctx.enter_context(tc.tile_pool(name="score", bufs=3))
        smpool = ctx.enter_context(tc.tile_pool(name="scorem", bufs=2))
        jpool = ctx.enter_context(tc.tile_pool(name="junk", bufs=2))
        ps_m = ctx.enter_context(tc.tile_pool(name="ps_m", bufs=2, space="PSUM"))
        ps_s = ctx.enter_context(tc.tile_pool(name="ps_s", bufs=1, space="PSUM"))
        ps_c = ctx.enter_context(tc.tile_pool(name="ps_c", bufs=3, space="PSUM"))

        mT = P.tile([L_DIM, b_loc], F32, tag="mT")
        s_colT = P.tile([SPC, NCHUNK], F32, tag="scolT")
        saT = P.tile([128, b_loc], F32, tag="saT")

        # filled by the phase-A tail (emitted mid-stream)
        gT = [P.tile([128, b_loc], F32, tag=f"gT{g}", name=f"gT{g}")
              for g in range(G_DIM // 128)]
        WwT = [P.tile([128, HID], F32, tag=f"WwT{g}", name=f"WwT{g}")
               for g in range(G_DIM // 128)]
        UwT = P.tile([L_DIM, HID], F32, tag="UwT")
        head_sb = []

        x_tiles = [None] * NCHUNK

        def transpose_to_sbuf(dst_ap, src_ap):
            pp, ff = src_ap.shape
            t_ps = ps_t.tile([128, 128], F32, tag="tps")
            nc.tensor.transpose(t_ps[0:ff, 0:pp], src_ap, ident[0:pp, 0:pp])
            nc.scalar.copy(dst_ap, t_ps[0:ff, 0:pp])

        def issue_dma(ch):
            x_ch = xpool.tile([128, NTOK], F32, tag="xch")
            src = ls[ch * NTOK:(ch + 1) * NTOK, :]
            dma_eng = nc.sync if ch % 2 == 0 else nc.scalar
            dma_eng.dma_start(
                x_ch[:, :], src.rearrange("(p j) d -> p (j d)", p=128))
            x_tiles[ch] = x_ch

        def compute_chunk(ch):
            x_ch = x_tiles[ch]
            # boundary-only cumsum(x * v): inner write stride 0, so each
            # 128-segment's final running sum lands at cum[:, 1+j].
            cum = cumpool.tile([128, 32], F32, tag="cum")
            nc.gpsimd.memset(cum[:, 0:1], 0.0)
            nc.vector._custom_dve(
                CUMSUM_MUL,
                out=cum[:, 1:J + 1, None].broadcast_to((128, J, 128)),
                in0=x_ch[:, :].rearrange("p (j d) -> p j d", d=128),
                in1=v_flat[:, :].rearrange("p (j d) -> p j d", d=128))

            # score16[p, j] = fp16(lrelu(E[j+1] - E[j] + c))
            score16 = scpool.tile([128, J], F16, tag="sc")
            nc.vector._custom_dve(SCORE_OP, out=score16[:, :],
                                  in0=cum[:, 1:J + 1],
                                  in1=cum[:, 0:J],
                                  s0=c_all[:, ch:ch + 1], s1=0.01)

            # scorem[p, (j,s)] = score16[p,j] * m16[p,s]
            scorem = smpool.tile([128, J * SPC], F16, tag="sm")
            sm3 = scorem[:, :].rearrange("p (j s) -> p j s", s=SPC)
            nc.vector.tensor_tensor(
                out=sm3[:, :, :],
                in0=score16[:, :, None].broadcast_to((128, J, SPC)),
                in1=m16_sb[:, None, :].broadcast_to((128, J, SPC)),
                op=AF.mult)

            # mT_chunk [128 feat, 16] accumulated over the 25 j-tiles;
            # lhsT is the bf16 high-halfword view of the fp32 x tile.
            mT_ps = ps_m.tile([L_DIM, SPC], F32, tag="mps")
            for j in range(J):
                xbj = x_ch[:, j * 128:(j + 1) * 128].bitcast(BF16)[:, 1:256:2]
                nc.tensor.matmul(out=mT_ps[:, :],
                                 lhsT=xbj,
                                 rhs=scorem[:, j * SPC:(j + 1) * SPC],
                                 start=(j == 0), stop=(j == J - 1))
            nc.scalar.copy(mT[:, ch * SPC:(ch + 1) * SPC], mT_ps[:, :])

            # s[s] = sum_{p,j} score16[p,j]*ind(p//8==s)  -> s_colT[:, ch]
            s_ps = ps_s.tile([SPC, J], F32, tag="sps")
            nc.tensor.matmul(out=s_ps[:, :], lhsT=m16_sb[:, :],
                             rhs=score16[:, :])
            junk_s = jpool.tile([SPC, J], F32, tag="jks")
            nc.scalar.activation(junk_s[:, :], s_ps[:, :], IDENT,
                                 accum_out=s_colT[:, ch:ch + 1])

        def phase_a_tail():
            for bb in range(NB):
                for g in range(G_DIM // 128):
                    transpose_to_sbuf(
                        gT[g][:, bb * 128:(bb + 1) * 128],
                        g_all[:, bb * G_DIM + g * 128:bb * G_DIM + (g + 1) * 128])
            for g in range(G_DIM // 128):
                transpose_to_sbuf(WwT[g][:, :], Ww_sb[:, g * 128:(g + 1) * 128])
            transpose_to_sbuf(UwT[:, :], Uw_sb[:, :])
            for bb in range(NB):
                transpose_to_sbuf(
                    saT[2 * HID:2 * HID + A_DIM, bb * 128:(bb + 1) * 128],
                    a_all[:, bb * A_DIM:(bb + 1) * A_DIM])
            for hh, (w1n, w2n, w3T, b1c, b2c, b3c) in enumerate(head_dma):
                w1T = P.tile([128, 256], F32, tag=f"w1T{hh}")
                for rh in range(2):
                    transpose_to_sbuf(w1T[:, rh * 128:(rh + 1) * 128],
                                      w1n[:, rh * 128:(rh + 1) * 128])
                w2T = [P.tile([128, 256], F32, tag=f"w2T{hh}_{kh}",
                              name=f"w2T{hh}_{kh}") for kh in range(2)]
                for rh in range(2):
                    for kh in range(2):
                        transpose_to_sbuf(
                            w2T[kh][:, rh * 128:(rh + 1) * 128],
                            w2n[:, rh * 256 + kh * 128:rh * 256 + (kh + 1) * 128])
                head_sb.append((w1T, w2T, w3T, b1c, b2c, b3c))

        def phase_c_block(bb):
            sl = slice(bb * 128, (bb + 1) * 128)
            ch0 = bb * CPB
            # wg block [32, 128] + sg row
            wg_ps = ps_c.tile([HID, 128], F32, tag="cps")
            for g in range(G_DIM // 128):
                nc.tensor.matmul(out=wg_ps[:, :], lhsT=WwT[g][:, :],
                                 rhs=gT[g][:, sl],
                                 start=(g == 0), stop=(g == G_DIM // 128 - 1))
            wg_sb = scratch.tile([HID, 128], F32, tag="wgsb")
            nc.scalar.activation(wg_sb[:, :], wg_ps[:, :], IDENT,
                                 bias=Wb_sb[:, :])
            sg_ps = ps_c.tile([1, 128], F32, tag="cps")
            nc.tensor.matmul(out=sg_ps[:, :], lhsT=a12[:, :], rhs=wg_sb[:, :])
            sg_lin = scratch.tile([1, 128], F32, tag="sglin")
            nc.scalar.activation(sg_lin[:, :], sg_ps[:, :], IDENT,
                                 bias=attb_sb[:, :])
            sg_raw = scratch.tile([1, 128], F32, tag="sgraw")
            nc.vector.scalar_tensor_tensor(out=sg_raw[:, :], in0=sg_lin[:, :],
                                           scalar=0.01, in1=sg_lin[:, :],
                                           op0=AF.mult, op1=AF.max)

            # s_row block [1, 128]: sample ch*16+s -> col (ch-ch0)*16+s
            srow_ps = ps_c.tile([1, 128], F32, tag="cps")
            for s in range(SPC):
                nc.tensor.matmul(out=srow_ps[0:1, s * CPB:(s + 1) * CPB],
                                 lhsT=ident[0:SPC, s:s + 1],
                                 rhs=s_colT[:, ch0:ch0 + CPB],
                                 skip_group_check=True)
            s_row = scratch.tile([1, 128], F32, tag="srow")
            nc.scalar.copy(
                s_row[0:1, :].rearrange("one (c s) -> one c s", s=SPC),
                srow_ps[0:1, :].rearrange("one (s c) -> one c s", s=SPC))

            total = scratch.tile([1, 128], F32, tag="total")
            nc.vector.tensor_tensor(out=total[:, :], in0=sg_raw[:, :],
                                    in1=s_row[:, :], op=AF.add)
            recip = scratch.tile([1, 128], F32, tag="recip")
            nc.vector.reciprocal_approx_fast(recip[:, :], total[:, :])
            gn_row = scratch.tile([1, 128], F32, tag="gn")
            nc.vector.tensor_tensor(out=gn_row[:, :], in0=sg_raw[:, :],
                                    in1=recip[:, :], op=AF.mult)

            r32_ps = ps_c.tile([HID, 128], F32, tag="cps")
            nc.tensor.matmul(out=r32_ps[:, :], lhsT=ones_row[0:1, 0:HID],
                             rhs=recip[:, :])
            r32 = scratch.tile([HID, 128], F32, tag="r32")
            nc.scalar.copy(r32[:, :], r32_ps[:, :])
            g32_ps = ps_c.tile([HID, 128], F32, tag="cps")
            nc.tensor.matmul(out=g32_ps[:, :], lhsT=ones_row[0:1, 0:HID],
                             rhs=gn_row[:, :])
            g32 = scratch.tile([HID, 128], F32, tag="g32")
            nc.scalar.copy(g32[:, :], g32_ps[:, :])

            lT_ps = ps_c.tile([HID, 128], F32, tag="cps")
            nc.tensor.matmul(out=lT_ps[:, :], lhsT=UwT[:, :], rhs=mT[:, sl],
                             start=True, stop=False)
            nc.tensor.matmul(out=lT_ps[:, :], lhsT=Ub_row[:, :], rhs=s_row[:, :],
                             start=False, stop=True)

            lnorm = scratch.tile([HID, 128], F32, tag="lnorm")
            nc.vector.tensor_tensor(out=lnorm[:, :], in0=lT_ps[:, :],
                                    in1=r32[:, :], op=AF.mult)
            gpart = scratch.tile([HID, 128], F32, tag="gpart")
            nc.vector.tensor_tensor(out=gpart[:, :], in0=wg_sb[:, :],
                                    in1=g32[:, :], op=AF.mult)
            nc.scalar.activation(saT[0:HID, sl], gpart[:, :], RELU)
            nc.scalar.activation(saT[HID:2 * HID, sl], lnorm[:, :], RELU)

            for h, (w1T, w2T, w3T, b1c, b2c, b3c) in enumerate(head_sb):
                h1 = []
                for rh in range(2):
                    h_ps = ps_c.tile([128, 128], F32, tag="cps")
                    nc.tensor.matmul(out=h_ps[:, :],
                                     lhsT=w1T[:, rh * 128:(rh + 1) * 128],
                                     rhs=saT[:, sl])
                    h_sb = scratch.tile([128, 128], F32, tag="h1sb")
                    nc.scalar.activation(h_sb[:, :], h_ps[:, :], RELU,
                                         bias=b1c[:, rh:rh + 1])
                    h1.append(h_sb)
                h2 = []
                for rh in range(2):
                    h_ps = ps_c.tile([128, 128], F32, tag="cps")
                    for kh in range(2):
                        nc.tensor.matmul(out=h_ps[:, :],
                                         lhsT=w2T[kh][:, rh * 128:(rh + 1) * 128],
                                         rhs=h1[kh][:, :],
                                         start=(kh == 0), stop=(kh == 1))
                    h_sb = scratch.tile([128, 128], F32, tag="h2sb")
                    nc.scalar.activation(h_sb[:, :], h_ps[:, :], RELU,
                                         bias=b2c[:, rh:rh + 1])
                    h2.append(h_sb)
                q_ps = ps_c.tile([1, 128], F32, tag="cps")
                for kh in range(2):
                    nc.tensor.matmul(out=q_ps[:, :], lhsT=w3T[:, kh:kh + 1],
                                     rhs=h2[kh][:, :],
                                     start=(kh == 0), stop=(kh == 1))
                q_row = scratch.tile([1, 128], F32, tag="qrow")
                nc.scalar.activation(q_row[:, :], q_ps[:, :], IDENT,
                                     bias=b3c[:, :])
                nc.sync.dma_start(out_d[h:h + 1, sl], q_row[:, :])

        # ---------------- stream ----------------
        for ch in range(PF):
            issue_dma(ch)
        for ch in range(NCHUNK):
            if ch + PF < NCHUNK:
                issue_dma(ch + PF)
            compute_chunk(ch)
            if ch == TAIL_CH:
                phase_a_tail()
            if ch > CPB and (ch - 1) % CPB == 0:
                phase_c_block((ch - 1) // CPB - 1)
        phase_c_block(NB - 1)

    nc.compile()
    return nc


def _make_m16():
    m = np.zeros((128, SPC), np.float16)
    for p in range(128):
        m[p, p // PPS] = 1.0
    return m


def _make_esel():
    e = np.zeros((128, PPS * 128), np.float32)
    for r in range(PPS):
        for p in range(128):
            e[r * SPC + p // PPS, r * 128 + p] = 1.0
    return e


def _shard_inputs(inputs, b_loc=B_LOC):
    """Full inputs -> list of per-core in_maps."""
    m16 = _make_m16()
    esel = _make_esel()
    maps = []
    for c in range(NCORES):
        sl = slice(c * b_loc, (c + 1) * b_loc)
        m = {
            "local_states": np.ascontiguousarray(
                inputs["local_states"][sl].reshape(b_loc * L, L_DIM)),
            "global_states": np.ascontiguousarray(inputs["global_states"][sl]),
            "actions": np.ascontiguousarray(inputs["actions"][sl]),
            "m16": m16,
            "esel": esel,
        }
        for k in ("W_w", "W_b", "U_w", "U_b", "att_b",
                  "l1_w", "l1_b", "l2_w", "l2_b", "l3_w", "l3_b",
                  "l4_w", "l4_b", "l5_w", "l5_b", "l6_w", "l6_b"):
            m[k] = np.ascontiguousarray(np.asarray(inputs[k], np.float32))
        m["att_w"] = np.ascontiguousarray(
            np.asarray(inputs["att_w"], np.float32).reshape(1, 2 * HID))
        maps.append(m)
    return maps


_CACHE = {}


def kernel(**inputs) -> np.ndarray:
    from concourse.bass_utils import run_bass_kernel_spmd

    inputs = {k: np.asarray(v, np.float32) for k, v in inputs.items()}
    if "nc" not in _CACHE:
        _CACHE["nc"] = build_bass()
    nc = _CACHE["nc"]
    maps = _shard_inputs(inputs)
    res = run_bass_kernel_spmd(nc, maps, list(range(NCORES)))
    outs = [res.results[c]["out"] for c in range(NCORES)]  # each [2, B_LOC]
    q = np.concatenate(outs, axis=1)  # [2, B]
    return q.reshape(2, B, 1).astype(np.float32)


# revision 27
# speedup vs baseline: 1.0989x; 1.0141x over previous
"""Trainium2 Bass kernel for the Critic (gnn_message_passing) problem.

Math (per sample b):
  wg   = W_w @ g + W_b                                  [32]
  score_l = lrelu(x_l . v + c_b)   with v = U_w^T a2,
        c_b = a1.wg + att_b + U_b.a2
  score_g = lrelu((a1+a2).wg + att_b)
  total = score_g + sum_l score_l
  l_part = (U_w @ m_b + U_b * s_b) / total,  m_b = sum_l score_l x_l
  g_part = (score_g / total) * wg
  sa = [relu(g_part); relu(l_part); action]
  q_h = l3 @ relu(l2 @ relu(l1 @ sa + b1) + b2) + b3   (two heads)

Layout (per core, pure data parallel x8, B_LOC = 512 samples):
  - local_states streamed in 32 fp32 chunks [128 part, 25*128]; partition p
    holds 25 consecutive tokens of sample p//8. Chunk loads alternate
    between the two HWDGE rings (qSP / qAct), software-pipelined PF chunks
    ahead; setup loads are batched into few large DMAs so dispatch never
    congests the rings.
  - t = x.v via ONE custom DVE op per chunk: a scan(ADD, Src0*Src1)
    cumulative sum whose OUTPUT access pattern has inner stride 0 over
    each 128-elem segment, so only each segment's last value (the
    per-token dot product boundary) lands in a tiny [128, 26] tile.
    A second custom op computes score16 = fp16(lrelu(E[j+1]-E[j]+c)).
  - NO fp16 copy of x: the PE m-pass reads x as bf16 bitcast views
    (high halfword of each fp32 = truncated bf16), stride-2 APs.
    Validated: score fp16 + x bf16-trunc -> ~9.4e-3 final rel err.
  - scorem = score16 (x) m16 on DVE (GPSIMD stalls behind DVE's SBUF
    port during 2-src custom ops, so GPSIMD is kept out of the stream).
  - m accumulated on PE (25 matmuls of [128,16] into PSUM per chunk);
    s via one m16-stationary matmul + ACT accumulate.
  - Transposes for phase C are emitted mid-stream (after chunk 5); phase C
    (wg/sg + combine + head MLPs) runs per 128-sample block inside the
    stream, since a block's normalization only needs its own 8 chunks.
"""
import os
import sys

sys.path.insert(0, "/opt/trn_rl_repo")

from contextlib import ExitStack

import numpy as np

import concourse.bass as bass
import concourse.tile as tile
from concourse import bacc
from concourse import mybir
from concourse import dve_ops as DO
from concourse.dve_ops import TENSOR_TENSOR_REDUCE as CUSTOM_TTR
from concourse.dve_spec import (Spec, Src0, Src1, C0, C1, lower, AluOp, scan,
                                maxx, _has_src1)
from concourse.dve_uop import DveOpSpec

F32 = mybir.dt.float32
F16 = mybir.dt.float16
BF16 = mybir.dt.bfloat16
AF = mybir.AluOpType
IDENT = mybir.ActivationFunctionType.Identity
RELU = mybir.ActivationFunctionType.Relu

G_DIM, L_DIM, A_DIM, HID = 256, 128, 64, 32
B, L = 4096, 200
NCORES = 8
B_LOC = B // NCORES          # 512 samples per core
J = 25                       # tokens per partition per chunk
SPC = 16                     # samples per chunk (128 partitions / 8 per sample)
PPS = L // J                 # partitions per sample = 8
NCHUNK = B_LOC // SPC        # 32 chunks
NB = B_LOC // 128            # 128-sample blocks
CPB = NCHUNK // NB           # chunks per block = 8
NTOK = J * 128               # free-dim elements per chunk (3200)
PF = 7                       # chunk DMA prefetch distance
XBUFS = 10                   # x_ch ring depth (>= PF + 2)
TAIL_CH = 4                  # emit phase-A-tail transposes after this chunk


def _register_dve_op(name, spec, subdim=False):
    if name in DO._SUB_OPCODE_FOR_NAME:
        return next(op for op in DO.OPS if op.name == name)
    row = DO._CUSTOM_DVE_ROW_BASE + len(DO.OPS)
    assert row < 0x20
    DO._SUB_OPCODE_FOR_NAME[name] = row
    shas = {}
    for ver in ("v3", "v4"):
        shas[ver] = DveOpSpec(name=name, opcode=row, uops=lower(spec, ver=ver),
                              rd1_en=_has_src1(spec)).sha(ver)
    op = DO.DveOp(name, spec, subdim=subdim, uops_sha=shas)
    DO.OPS.append(op)
    DO.CUSTOM_DVE_SPECS[name] = spec
    return op


def _ref_cumsum_mul(in0, in1, c0, c1, c2):
    return np.cumsum(in0.astype(np.float32) * np.asarray(in1, np.float32),
                     axis=-1, dtype=np.float32)


def _ref_score(in0, in1, c0, c1, c2):
    dd = in0.astype(np.float32) - np.asarray(in1, np.float32) + c0
    return np.maximum(dd, dd * c1)


CUMSUM_MUL = _register_dve_op(
    "CUMSUM_MUL_ANT",
    Spec(body=scan(AluOp.ADD, Src0 * Src1), reference=_ref_cumsum_mul))

SCORE_OP = _register_dve_op(
    "DIFF_BIAS_LRELU_ANT",
    Spec(body=maxx((Src0 - Src1) + C0, ((Src0 - Src1) + C0) * C1),
         reference=_ref_score))


def build_bass(b_loc=B_LOC):
    tok = b_loc * L
    nc = bacc.Bacc()

    ls = nc.dram_tensor("local_states", [tok, L_DIM], F32, kind="ExternalInput")
    gs = nc.dram_tensor("global_states", [b_loc, G_DIM], F32, kind="ExternalInput")
    ac = nc.dram_tensor("actions", [b_loc, A_DIM], F32, kind="ExternalInput")
    Ww = nc.dram_tensor("W_w", [HID, G_DIM], F32, kind="ExternalInput")
    Wb = nc.dram_tensor("W_b", [HID], F32, kind="ExternalInput")
    Uw = nc.dram_tensor("U_w", [HID, L_DIM], F32, kind="ExternalInput")
    Ub = nc.dram_tensor("U_b", [HID], F32, kind="ExternalInput")
    attw = nc.dram_tensor("att_w", [1, 2 * HID], F32, kind="ExternalInput")
    attb = nc.dram_tensor("att_b", [1], F32, kind="ExternalInput")
    heads = []
    for h, names in enumerate((("l1", "l2", "l3"), ("l4", "l5", "l6"))):
        w1 = nc.dram_tensor(f"{names[0]}_w", [256, 128], F32, kind="ExternalInput")
        b1 = nc.dram_tensor(f"{names[0]}_b", [256], F32, kind="ExternalInput")
        w2 = nc.dram_tensor(f"{names[1]}_w", [256, 256], F32, kind="ExternalInput")
        b2 = nc.dram_tensor(f"{names[1]}_b", [256], F32, kind="ExternalInput")
        w3 = nc.dram_tensor(f"{names[2]}_w", [1, 256], F32, kind="ExternalInput")
        b3 = nc.dram_tensor(f"{names[2]}_b", [1], F32, kind="ExternalInput")
        heads.append((w1, b1, w2, b2, w3, b3))
    m16_d = nc.dram_tensor("m16", [128, SPC], F16, kind="ExternalInput")
    esel_d = nc.dram_tensor("esel", [128, PPS * 128], F32, kind="ExternalInput")
    out_d = nc.dram_tensor("out", [2, b_loc], F32, kind="ExternalOutput")

    with tile.TileContext(nc) as tc, ExitStack() as ctx:
        P = ctx.enter_context(tc.tile_pool(name="persist", bufs=1))
        scratch = ctx.enter_context(tc.tile_pool(name="scratch", bufs=2))
        ps_t = ctx.enter_context(tc.tile_pool(name="ps_t", bufs=2, space="PSUM"))

        # ---------------- Phase A: setup --------------------------------
        from concourse.masks import make_identity

        ident = P.tile([128, 128], F32, tag="ident")
        make_identity(nc, ident[:, :])
        ones_row = P.tile([1, 128], F32, tag="onesr")
        nc.vector.memset(ones_row[:, :], 1.0)

        # --- setup DMAs, sync ring (small weights + globals) ---
        m16_sb = P.tile([128, SPC], F16, tag="m16")
        nc.sync.dma_start(m16_sb[:, :], m16_d[:, :])
        esel = P.tile([128, PPS * 128], F32, tag="esel")
        nc.sync.dma_start(esel[:, :], esel_d[:, :])
        Ww_sb = P.tile([HID, G_DIM], F32, tag="Ww")
        nc.sync.dma_start(Ww_sb[:, :], Ww[:, :])
        Wb_sb = P.tile([HID, 1], F32, tag="Wb")
        nc.sync.dma_start(Wb_sb[:, :], Wb[:][:, None])
        Uw_sb = P.tile([HID, L_DIM], F32, tag="Uw")
        nc.sync.dma_start(Uw_sb[:, :], Uw[:, :])
        Ub_col = P.tile([HID, 1], F32, tag="Ubc")
        nc.sync.dma_start(Ub_col[:, :], Ub[:][:, None])
        Ub_row = P.tile([1, HID], F32, tag="Ubr")
        nc.sync.dma_start(Ub_row[:, :], Ub[:][None, :])
        a1_sb = P.tile([HID, 1], F32, tag="a1")
        nc.sync.dma_start(a1_sb[:, :], attw[0, 0:HID][:, None])
        a2_sb = P.tile([HID, 1], F32, tag="a2")
        nc.sync.dma_start(a2_sb[:, :], attw[0, HID:2 * HID][:, None])
        attb_sb = P.tile([1, 1], F32, tag="attb")
        nc.sync.dma_start(attb_sb[:, :], attb[:][None, :])
        # all 512 global states in one DMA: partition p <- sample bb*128+p
        g_all = P.tile([128, NB * G_DIM], F32, tag="gall")
        nc.sync.dma_start(
            g_all[:, :].rearrange("p (bb g) -> p bb g", g=G_DIM),
            gs[:, :].rearrange("(bb p) g -> p bb g", p=128))

        # --- setup DMAs, scalar ring (head weights + actions, batched) ---
        head_dma = []
        for hh, (w1, b1, w2, b2, w3, b3) in enumerate(heads):
            w1n = P.tile([128, 256], F32, tag=f"w1n{hh}")
            nc.scalar.dma_start(
                w1n[:, :].rearrange("p (r d) -> p r d", d=128),
                w1[:, :].rearrange("(r p) d -> p r d", p=128))
            w2n = P.tile([128, 512], F32, tag=f"w2n{hh}")
            nc.scalar.dma_start(
                w2n[:, :].rearrange("p (r d) -> p r d", d=256),
                w2[:, :].rearrange("(r p) d -> p r d", p=128))
            w3T = P.tile([128, 2], F32, tag=f"w3T{hh}")
            nc.scalar.dma_start(w3T[:, :],
                                w3[0, :].rearrange("(k p) -> p k", p=128))
            b1c = P.tile([128, 2], F32, tag=f"b1c{hh}")
            nc.scalar.dma_start(b1c[:, :],
                                b1[:].rearrange("(r p) -> p r", p=128))
            b2c = P.tile([128, 2], F32, tag=f"b2c{hh}")
            nc.scalar.dma_start(b2c[:, :],
                                b2[:].rearrange("(r p) -> p r", p=128))
            b3c = P.tile([1, 1], F32, tag=f"b3c{hh}")
            nc.scalar.dma_start(b3c[:, :], b3[:][None, :])
            head_dma.append((w1n, w2n, w3T, b1c, b2c, b3c))
        a_all = P.tile([128, NB * A_DIM], F32, tag="aall")
        nc.scalar.dma_start(
            a_all[:, :].rearrange("p (bb a) -> p bb a", a=A_DIM),
            ac[:, :].rearrange("(bb p) a -> p bb a", p=128))

        # --- v_flat [128, 3200] fp32 (PE + DVE copies only) ---
        v_ps = ps_t.tile([1, L_DIM], F32, tag="tps")
        nc.tensor.matmul(out=v_ps[:, :], lhsT=a2_sb[:, :], rhs=Uw_sb[:, :])
        v_row = P.tile([1, L_DIM], F32, tag="vrow")
        nc.vector.tensor_copy(v_row[:, :], v_ps[:, :])
        vrep_ps = ps_t.tile([128, 128], F32, tag="tps")
        nc.tensor.matmul(out=vrep_ps[:, :], lhsT=ones_row[:, :], rhs=v_row[:, :])
        v_flat = P.tile([128, NTOK], F32, tag="vflat")
        nc.vector.tensor_copy(v_flat[:, 0:128], vrep_ps[:, :])
        filled = 128
        while filled < NTOK:
            n = min(filled, NTOK - filled)
            nc.vector.tensor_copy(v_flat[:, filled:filled + n], v_flat[:, 0:n])
            filled += n

        # --- c_all [128, NCHUNK] ---
        u_ps = ps_t.tile([1, G_DIM], F32, tag="tps")
        nc.tensor.matmul(out=u_ps[:, :], lhsT=a1_sb[:, :], rhs=Ww_sb[:, :])
        u_row = P.tile([1, G_DIM], F32, tag="urow")
        nc.vector.tensor_copy(u_row[:, :], u_ps[:, :])
        urep_ps = ps_t.tile([128, G_DIM], F32, tag="tps")
        nc.tensor.matmul(out=urep_ps[:, :], lhsT=ones_row[:, :], rhs=u_row[:, :])
        u_rep = P.tile([128, G_DIM], F32, tag="urep")
        nc.vector.tensor_copy(u_rep[:, :], urep_ps[:, :])
        # c_col4 off the DVE: multiply on GPSIMD, reduce via ACT accumulate
        # (both engines idle pre-stream; shortens the DVE queue before cum0)
        c_col4 = P.tile([128, NB], F32, tag="ccol4")
        for bb in range(NB):
            junkA = scratch.tile([128, G_DIM], F32, tag="junkA",
                                 name=f"junkA{bb}")
            nc.gpsimd.tensor_tensor(
                out=junkA[:, :],
                in0=g_all[:, bb * G_DIM:(bb + 1) * G_DIM],
                in1=u_rep[:, :], op=AF.mult)
            junkB = scratch.tile([128, G_DIM], F32, tag="junkB",
                                 name=f"junkB{bb}")
            nc.scalar.activation(junkB[:, :], junkA[:, :], IDENT,
                                 accum_out=c_col4[:, bb:bb + 1])
        uba2_ps = ps_t.tile([1, 1], F32, tag="tps")
        nc.tensor.matmul(out=uba2_ps[:, :], lhsT=Ub_col[:, :], rhs=a2_sb[:, :],
                         start=True, stop=False, skip_group_check=True)
        nc.tensor.matmul(out=uba2_ps[:, :], lhsT=Wb_sb[:, :], rhs=a1_sb[:, :],
                         start=False, stop=True, skip_group_check=True)
        cconst = P.tile([1, 1], F32, tag="cconst")
        nc.vector.tensor_tensor(out=cconst[:, :], in0=uba2_ps[:, :],
                                in1=attb_sb[:, :], op=AF.add)
        cc128_ps = ps_t.tile([128, 1], F32, tag="tps")
        nc.tensor.matmul(out=cc128_ps[:, :], lhsT=ones_row[0:1, :],
                         rhs=cconst[:, :])
        cc128 = P.tile([128, 1], F32, tag="cc128")
        nc.vector.tensor_copy(cc128[:, :], cc128_ps[:, :])
        call_ps = ps_t.tile([128, NCHUNK], F32, tag="tps")
        for r in range(PPS):
            nc.tensor.matmul(out=call_ps[:, r:NCHUNK:PPS],
                             lhsT=esel[:, r * 128:(r + 1) * 128],
                             rhs=c_col4[:, :], skip_group_check=True)
        c_all = P.tile([128, NCHUNK], F32, tag="call")
        nc.scalar.activation(c_all[:, :], call_ps[:, :], IDENT,
                             bias=cc128[:, :])

        a12 = P.tile([HID, 1], F32, tag="a12")
        nc.vector.tensor_tensor(out=a12[:, :], in0=a1_sb[:, :], in1=a2_sb[:, :],
                                op=AF.add)

        # ---------------- pools for stream + phase C ----------------
        xpool = ctx.enter_context(tc.tile_pool(name="xchunk", bufs=XBUFS))
        cumpool = ctx.enter_context(tc.tile_pool(name="cump", bufs=3))
        scpool = ctx.enter_context(tc.tile_pool(name="score", bufs=3))
        smpool = ctx.enter_context(tc.tile_pool(name="scorem", bufs=2))
        jpool = ctx.enter_context(tc.tile_pool(name="junk", bufs=2))
        ps_m = ctx.enter_context(tc.tile_pool(name="ps_m", bufs=2, space="PSUM"))
        ps_s = ctx.enter_context(tc.tile_pool(name="ps_s", bufs=1, space="PSUM"))
        ps_c = ctx.enter_context(tc.tile_pool(name="ps_c", bufs=3, space="PSUM"))

        mT = P.tile([L_DIM, b_loc], F32, tag="mT")
        s_colT = P.tile([SPC, NCHUNK], F32, tag="scolT")
        saT = P.tile([128, b_loc], F32, tag="saT")

        # filled by the phase-A tail (emitted mid-stream)
        gT = [P.tile([128, b_loc], F32, tag=f"gT{g}", name=f"gT{g}")
              for g in range(G_DIM // 128)]
        WwT = [P.tile([128, HID], F32, tag=f"WwT{g}", name=f"WwT{g}")
               for g in range(G_DIM // 128)]
        UwT = P.tile([L_DIM, HID], F32, tag="UwT")
        head_sb = []

        x_tiles = [None] * NCHUNK

        def transpose_to_sbuf(dst_ap, src_ap):
            pp, ff = src_ap.shape
            t_ps = ps_t.tile([128, 128], F32, tag="tps")
            nc.tensor.transpose(t_ps[0:ff, 0:pp], src_ap, ident[0:pp, 0:pp])
            nc.scalar.copy(dst_ap, t_ps[0:ff, 0:pp])

        def issue_dma(ch):
            x_ch = xpool.tile([128, NTOK], F32, tag="xch")
            src = ls[ch * NTOK:(ch + 1) * NTOK, :]
            dma_eng = nc.sync if ch % 2 == 0 else nc.scalar
            dma_eng.dma_start(
                x_ch[:, :], src.rearrange("(p j) d -> p (j d)", p=128))
            x_tiles[ch] = x_ch

        def compute_chunk(ch):
            x_ch = x_tiles[ch]
            # boundary-only cumsum(x * v): inner write stride 0, so each
            # 128-segment's final running sum lands at cum[:, 1+j].
            cum = cumpool.tile([128, 32], F32, tag="cum")
            nc.gpsimd.memset(cum[:, 0:1], 0.0)
            nc.vector._custom_dve(
                CUMSUM_MUL,
                out=cum[:, 1:J + 1, None].broadcast_to((128, J, 128)),
                in0=x_ch[:, :].rearrange("p (j d) -> p j d", d=128),
                in1=v_flat[:, :].rearrange("p (j d) -> p j d", d=128))

            # score16[p, j] = fp16(lrelu(E[j+1] - E[j] + c))
            score16 = scpool.tile([128, J], F16, tag="sc")
            nc.vector._custom_dve(SCORE_OP, out=score16[:, :],
                                  in0=cum[:, 1:J + 1],
                                  in1=cum[:, 0:J],
                                  s0=c_all[:, ch:ch + 1], s1=0.01)

            # scorem[p, (j,s)] = score16[p,j] * m16[p,s]
            scorem = smpool.tile([128, J * SPC], F16, tag="sm")
            sm3 = scorem[:, :].rearrange("p (j s) -> p j s", s=SPC)
            nc.vector.tensor_tensor(
                out=sm3[:, :, :],
                in0=score16[:, :, None].broadcast_to((128, J, SPC)),
                in1=m16_sb[:, None, :].broadcast_to((128, J, SPC)),
                op=AF.mult)

            # mT_chunk [128 feat, 16] accumulated over the 25 j-tiles;
            # lhsT is the bf16 high-halfword view of the fp32 x tile.
            mT_ps = ps_m.tile([L_DIM, SPC], F32, tag="mps")
            for j in range(J):
                xbj = x_ch[:, j * 128:(j + 1) * 128].bitcast(BF16)[:, 1:256:2]
                nc.tensor.matmul(out=mT_ps[:, :],
                                 lhsT=xbj,
                                 rhs=scorem[:, j * SPC:(j + 1) * SPC],
                                 start=(j == 0), stop=(j == J - 1))
            nc.scalar.copy(mT[:, ch * SPC:(ch + 1) * SPC], mT_ps[:, :])

            # s[s] = sum_{p,j} score16[p,j]*ind(p//8==s)  -> s_colT[:, ch]
            s_ps = ps_s.tile([SPC, J], F32, tag="sps")
            nc.tensor.matmul(out=s_ps[:, :], lhsT=m16_sb[:, :],
                             rhs=score16[:, :])
            junk_s = jpool.tile([SPC, J], F32, tag="jks")
            nc.scalar.activation(junk_s[:, :], s_ps[:, :], IDENT,
                                 accum_out=s_colT[:, ch:ch + 1])

        def phase_a_tail():
            for bb in range(NB):
                for g in range(G_DIM // 128):
                    transpose_to_sbuf(
                        gT[g][:, bb * 128:(bb + 1) * 128],
                        g_all[:, bb * G_DIM + g * 128:bb * G_DIM + (g + 1) * 128])
            for g in range(G_DIM // 128):
                transpose_to_sbuf(WwT[g][:, :], Ww_sb[:, g * 128:(g + 1) * 128])
            transpose_to_sbuf(UwT[:, :], Uw_sb[:, :])
            for bb in range(NB):
                transpose_to_sbuf(
                    saT[2 * HID:2 * HID + A_DIM, bb * 128:(bb + 1) * 128],
                    a_all[:, bb * A_DIM:(bb + 1) * A_DIM])
            for hh, (w1n, w2n, w3T, b1c, b2c, b3c) in enumerate(head_dma):
                w1T = P.tile([128, 256], F32, tag=f"w1T{hh}")
                for rh in range(2):
                    transpose_to_sbuf(w1T[:, rh * 128:(rh + 1) * 128],
                                      w1n[:, rh * 128:(rh + 1) * 128])
                w2T = [P.tile([128, 256], F32, tag=f"w2T{hh}_{kh}",
                              name=f"w2T{hh}_{kh}") for kh in range(2)]
                for rh in range(2):
                    for kh in range(2):
                        transpose_to_sbuf(
                            w2T[kh][:, rh * 128:(rh + 1) * 128],
                            w2n[:, rh * 256 + kh * 128:rh * 256 + (kh + 1) * 128])
                head_sb.append((w1T, w2T, w3T, b1c, b2c, b3c))

        def phase_c_block(bb):
            sl = slice(bb * 128, (bb + 1) * 128)
            ch0 = bb * CPB
            # wg block [32, 128] + sg row
            wg_ps = ps_c.tile([HID, 128], F32, tag="cps")
            for g in range(G_DIM // 128):
                nc.tensor.matmul(out=wg_ps[:, :], lhsT=WwT[g][:, :],
                                 rhs=gT[g][:, sl],
                                 start=(g == 0), stop=(g == G_DIM // 128 - 1))
            wg_sb = scratch.tile([HID, 128], F32, tag="wgsb")
            nc.scalar.activation(wg_sb[:, :], wg_ps[:, :], IDENT,
                                 bias=Wb_sb[:, :])
            sg_ps = ps_c.tile([1, 128], F32, tag="cps")
            nc.tensor.matmul(out=sg_ps[:, :], lhsT=a12[:, :], rhs=wg_sb[:, :])
            sg_lin = scratch.tile([1, 128], F32, tag="sglin")
            nc.scalar.activation(sg_lin[:, :], sg_ps[:, :], IDENT,
                                 bias=attb_sb[:, :])
            sg_raw = scratch.tile([1, 128], F32, tag="sgraw")
            nc.vector.scalar_tensor_tensor(out=sg_raw[:, :], in0=sg_lin[:, :],
                                           scalar=0.01, in1=sg_lin[:, :],
                                           op0=AF.mult, op1=AF.max)

            # s_row block [1, 128]: sample ch*16+s -> col (ch-ch0)*16+s
            srow_ps = ps_c.tile([1, 128], F32, tag="cps")
            for s in range(SPC):
                nc.tensor.matmul(out=srow_ps[0:1, s * CPB:(s + 1) * CPB],
                                 lhsT=ident[0:SPC, s:s + 1],
                                 rhs=s_colT[:, ch0:ch0 + CPB],
                                 skip_group_check=True)
            s_row = scratch.tile([1, 128], F32, tag="srow")
            nc.scalar.copy(
                s_row[0:1, :].rearrange("one (c s) -> one c s", s=SPC),
                srow_ps[0:1, :].rearrange("one (s c) -> one c s", s=SPC))

            total = scratch.tile([1, 128], F32, tag="total")
            nc.vector.tensor_tensor(out=total[:, :], in0=sg_raw[:, :],
                                    in1=s_row[:, :], op=AF.add)
            recip = scratch.tile([1, 128], F32, tag="recip")
            nc.vector.reciprocal_approx_fast(recip[:, :], total[:, :])
            gn_row = scratch.tile([1, 128], F32, tag="gn")
            nc.vector.tensor_tensor(out=gn_row[:, :], in0=sg_raw[:, :],
                                    in1=recip[:, :], op=AF.mult)

            r32_ps = ps_c.tile([HID, 128], F32, tag="cps")
            nc.tensor.matmul(out=r32_ps[:, :], lhsT=ones_row[0:1, 0:HID],
                             rhs=recip[:, :])
            r32 = scratch.tile([HID, 128], F32, tag="r32")
            nc.scalar.copy(r32[:, :], r32_ps[:, :])
            g32_ps = ps_c.tile([HID, 128], F32, tag="cps")
            nc.tensor.matmul(out=g32_ps[:, :], lhsT=ones_row[0:1, 0:HID],
                             rhs=gn_row[:, :])
            g32 = scratch.tile([HID, 128], F32, tag="g32")
            nc.scalar.copy(g32[:, :], g32_ps[:, :])

            lT_ps = ps_c.tile([HID, 128], F32, tag="cps")
            nc.tensor.matmul(out=lT_ps[:, :], lhsT=UwT[:, :], rhs=mT[:, sl],
                             start=True, stop=False)
            nc.tensor.matmul(out=lT_ps[:, :], lhsT=Ub_row[:, :], rhs=s_row[:, :],
                             start=False, stop=True)

            lnorm = scratch.tile([HID, 128], F32, tag="lnorm")
            nc.vector.tensor_tensor(out=lnorm[:, :], in0=lT_ps[:, :],
                                    in1=r32[:, :], op=AF.mult)
            gpart = scratch.tile([HID, 128], F32, tag="gpart")
            nc.vector.tensor_tensor(out=gpart[:, :], in0=wg_sb[:, :],
                                    in1=g32[:, :], op=AF.mult)
            nc.scalar.activation(saT[0:HID, sl], gpart[:, :], RELU)
            nc.scalar.activation(saT[HID:2 * HID, sl], lnorm[:, :], RELU)

            for h, (w1T, w2T, w3T, b1c, b2c, b3c) in enumerate(head_sb):
                h1 = []
                for rh in range(2):
                    h_ps = ps_c.tile([128, 128], F32, tag="cps")
                    nc.tensor.matmul(out=h_ps[:, :],
                                     lhsT=w1T[:, rh * 128:(rh + 1) * 128],
                                     rhs=saT[:, sl])
                    h_sb = scratch.tile([128, 128], F32, tag="h1sb")
                    nc.scalar.activation(h_sb[:, :], h_ps[:, :], RELU,
                                         bias=b1c[:, rh:rh + 1])
                    h1.append(h_sb)
                h2 = []
                for rh in range(2):
                    h_ps = ps_c.tile([128, 128], F32, tag="cps")
                    for kh in range(2):
                        nc.tensor.matmul(out=h_ps[:, :],
                                         lhsT=w2T[kh][:, rh * 128:(rh + 1) * 128],
                                         rhs=h1[kh][:, :],
                                         start=(kh == 0), stop=(kh == 1))
                    h_sb = scratch.tile([128, 128], F32, tag="h2sb")
                    nc.scalar.activation(h_sb[:, :], h_ps[:, :], RELU,
                                         bias=b2c[:, rh:rh + 1])
                    h2.append(h_sb)
                q_ps = ps_c.tile([1, 128], F32, tag="cps")
                for kh in range(2):
                    nc.tensor.matmul(out=q_ps[:, :], lhsT=w3T[:, kh:kh + 1],
                                     rhs=h2[kh][:, :],
                                     start=(kh == 0), stop=(kh == 1))
                q_row = scratch.tile([1, 128], F32, tag="qrow")
                nc.scalar.activation(q_row[:, :], q_ps[:, :], IDENT,
                                     bias=b3c[:, :])
                nc.sync.dma_start(out_d[h:h + 1, sl], q_row[:, :])

        # ---------------- stream ----------------
        for ch in range(PF):
            issue_dma(ch)
        for ch in range(NCHUNK):
            if ch + PF < NCHUNK:
                issue_dma(ch + PF)
            compute_chunk(ch)
            if ch == TAIL_CH:
                phase_a_tail()
            if ch > CPB and (ch - 1) % CPB == 0:
                phase_c_block((ch - 1) // CPB - 1)
        phase_c_block(NB - 1)

    nc.compile()
    return nc


def _make_m16():
    m = np.zeros((128, SPC), np.float16)
    for p in range(128):
        m[p, p // PPS] = 1.0
    return m


def _make_esel():
    e = np.zeros((128, PPS * 128), np.float32)
    for r in range(PPS):
        for p in range(128):
            e[r * SPC + p // PPS, r * 128 + p] = 1.0
    return e


def _shard_inputs(inputs, b_loc=B_LOC):
    """Full inputs -> list of per-core in_maps."""
    m16 = _make_m16()
    esel = _make_esel()
    maps = []
    for c in range(NCORES):
        sl = slice(c * b_loc, (c + 1) * b_loc)
        m = {
            "local_states": np.ascontiguousarray(
                inputs["local_states"][sl].reshape(b_loc * L, L_DIM)),
            "global_states": np.ascontiguousarray(inputs["global_states"][sl]),
            "actions": np.ascontiguousarray(inputs["actions"][sl]),
            "m16": m16,
            "esel": esel,
        }
        for k in ("W_w", "W_b", "U_w", "U_b", "att_b",
                  "l1_w", "l1_b", "l2_w", "l2_b", "l3_w", "l3_b",
                  "l4_w", "l4_b", "l5_w", "l5_b", "l6_w", "l6_b"):
            m[k] = np.ascontiguousarray(np.asarray(inputs[k], np.float32))
        m["att_w"] = np.ascontiguousarray(
            np.asarray(inputs["att_w"], np.float32).reshape(1, 2 * HID))
        maps.append(m)
    return maps


_CACHE = {}


def kernel(**inputs) -> np.ndarray:
    from concourse.bass_utils import run_bass_kernel_spmd

    inputs = {k: np.asarray(v, np.float32) for k, v in inputs.items()}
    if "nc" not in _CACHE:
        _CACHE["nc"] = build_bass()
    nc = _CACHE["nc"]
    maps = _shard_inputs(inputs)
    res = run_bass_kernel_spmd(nc, maps, list(range(NCORES)))
    outs = [res.results[c]["out"] for c in range(NCORES)]  # each [2, B_LOC]
    q = np.concatenate(outs, axis=1)  # [2, B]
    return q.reshape(2, B, 1).astype(np.float32)
